# revision 1
# baseline (speedup 1.0000x reference)
"""Trainium2 Bass kernel for CrossAttention (B=4, N=M=2048, H=8, D=64,
Q_DIM=1024, C_DIM=768).

Sharding over 8 cores: core c handles batch b = c//2 and head-group
hg = c%2 (4 heads, 256 inner dims).  Each core computes a *partial*
output projection (its 256 inner dims of the 512 total); the host sums
core pairs and adds the output bias.

Device-side layouts are all matmul-native (out = lhsT.T @ rhs):
  - qT/kT [inner, seq]: computed with weight-chunk stationary, xT/ctxT
    moving.  v [keys, inner] with ctxT-chunk stationary, WvT moving.
  - scores are computed transposed: S.T[keys, q] = kT_h contracted with
    qT_h over the head dim, so softmax's key-reduction is a matmul
    reduction and no on-chip transposes are needed anywhere.
  - V carries an appended ones-column, so the P.T@V matmul also yields the
    per-query softmax denominators (row 64 of the [65, q] accumulator).
  - denominators are broadcast across partitions with a K=1 ones matmul,
    reciprocal'd on VectorE, and folded into the PSUM->SBUF copy of the
    attention output.
  - all matmul inputs are float32r (single-pass FP22 multiply, fp32
    accumulate) for 4x tensor-engine throughput vs true fp32.

Schedule (this revision): a static schedule tuned against the concourse
timeline cost model.
  - DMA order puts the first head's inputs (wk, ctx keys 0-1023, wv,
    wq-m0, x0) first; the context is split into key-halves so h0's
    first 8 key-chunks run while the second half streams.
  - kT(K1, both m), v(kc0-7) and qT(m0,qb0) are computed in the
    prologue DMA shadow, paced by per-chunk DMA arrival.
  - attention runs as one flat lead-2 software pipeline over
    (block, head, key-chunk) steps: scores for step g+2 are emitted at
    step g, so the Act queue always holds two ready exps and the
    2-buf score ring throttles the PE to the exp chain's pace.
  - remaining projection work (kT K2 quarters, qT m1/qb1 parts, the
    previous block's final projections, v kc8-15) fills per-step
    slots in <=4-matmul pieces on the 2KB psB transient ring.
  - the tail normalizes the last head in 256-wide pieces and drains
    the final projections across all three free PSUM rings with
    evictions alternating Act/DVE; output DMAs issue from the idle
    SP queue.

The attention mask in this problem is all-True; if a mask with False
entries is ever passed, kernel() falls back to a numpy reference.
"""

import numpy as np

B, N, M = 4, 2048, 2048
Q_DIM, C_DIM, H, D = 1024, 768, 8, 64
INNER = H * D  # 512
SCALE = D ** -0.5

N_CORES = 8
H_PER_CORE = 4          # heads per core
IN_PER_CORE = H_PER_CORE * D  # 256 inner dims per core
QB = 1024               # query block
N_QB = N // QB          # 2
KC = M // 128           # 16 key chunks
QK_CHUNKS = Q_DIM // 128   # 8
CK_CHUNKS = C_DIM // 128   # 6
IN_CHUNKS = IN_PER_CORE // 128  # 2

_CACHED_NC = None


def _build_bass():
    import concourse.bass as bass
    import concourse.mybir as mybir
    import concourse.tile as tile
    from concourse import bacc

    f32r = mybir.dt.float32r
    f32 = mybir.dt.float32
    ts, ds = bass.ts, bass.ds
    Exp = mybir.ActivationFunctionType.Exp

    nc = bacc.Bacc("TRN2", target_bir_lowering=False)

    xT = nc.dram_tensor("xT", [Q_DIM, N], f32r, kind="ExternalInput")
    cT = nc.dram_tensor("cT", [C_DIM, M], f32r, kind="ExternalInput")
    wq = nc.dram_tensor("wq", [Q_DIM, IN_PER_CORE], f32r, kind="ExternalInput")
    wk = nc.dram_tensor("wk", [C_DIM, IN_PER_CORE], f32r, kind="ExternalInput")
    wv = nc.dram_tensor("wv", [C_DIM, IN_PER_CORE], f32r, kind="ExternalInput")
    wo = nc.dram_tensor("wo", [IN_PER_CORE, Q_DIM], f32r, kind="ExternalInput")
    out_d = nc.dram_tensor("out", [N, Q_DIM], f32, kind="ExternalOutput")

    with tile.TileContext(nc) as tc:
        with (
            tc.tile_pool(name="persist", bufs=1) as persist,
            tc.tile_pool(name="stream", bufs=2) as stream,
            tc.tile_pool(name="psA", bufs=2, space="PSUM") as psA,
            tc.tile_pool(name="psB", bufs=2, space="PSUM") as psB,
        ):
            wk_r = wk.rearrange("(k p) n -> p k n", p=128)
            wq_r = wq.rearrange("(k p) n -> p k n", p=128)

            # ---- constants ----
            onesf = persist.tile([128, D], f32, tag="onesf")
            nc.vector.memset(onesf, 1.0)
            ones_sb = persist.tile([D + 1, D], f32r, tag="ones")
            nc.vector.tensor_copy(out=ones_sb, in_=onesf[0 : D + 1, :])

            # ---- DMA schedule (execution order == emission order) ----
            wk_sb = persist.tile([128, CK_CHUNKS, IN_PER_CORE], f32r, tag="wk")
            nc.sync.dma_start(out=wk_sb, in_=wk_r)

            ctx_sb = persist.tile([128, CK_CHUNKS, M], f32r, tag="ctx")
            for c in range(CK_CHUNKS):
                nc.sync.dma_start(
                    out=ctx_sb[:, c, 0:1024], in_=cT[ds(c * 128, 128), 0:1024]
                )

            wv_sb = persist.tile([128, CK_CHUNKS, IN_PER_CORE], f32r, tag="wv")
            nc.sync.dma_start(out=wv_sb, in_=wv.rearrange("(k p) n -> p k n", p=128))

            wq_sb = persist.tile([128, QK_CHUNKS, IN_PER_CORE], f32r, tag="wq")
            nc.sync.dma_start(out=wq_sb[:, :, 0:128], in_=wq_r[:, :, 0:128])

            x0_sb = stream.tile([128, QK_CHUNKS, QB], f32r, tag="x", bufs=1,
                                name="x0")
            for k in range(QK_CHUNKS):
                nc.sync.dma_start(
                    out=x0_sb[:, k, :], in_=xT[ds(k * 128, 128), 0:QB]
                )

            nc.sync.dma_start(out=wq_sb[:, :, 128:256], in_=wq_r[:, :, 128:256])
            for c in range(CK_CHUNKS):
                nc.sync.dma_start(
                    out=ctx_sb[:, c, 1024:2048], in_=cT[ds(c * 128, 128), 1024:2048]
                )

            wo_sb = persist.tile([128, IN_CHUNKS, Q_DIM], f32r, tag="wo")
            nc.sync.dma_start(out=wo_sb, in_=wo.rearrange("(t p) n -> p t n", p=128))

            # x1 shares the x slot; its DMA waits on the slot-release sem at
            # runtime (qT qb1 reads pace with per-chunk arrival), so it is
            # last in the DMA program order.
            x1_sb = stream.tile([128, QK_CHUNKS, QB], f32r, tag="x", bufs=1,
                                name="x1")
            for k in range(QK_CHUNKS):
                nc.sync.dma_start(
                    out=x1_sb[:, k, :], in_=xT[ds(k * 128, 128), QB : 2 * QB]
                )

            # ---- persistent compute targets ----
            kT_sb = persist.tile([128, IN_CHUNKS, M], f32r, tag="kt")
            v_sb = persist.tile([128, KC, H_PER_CORE, D + 1], f32r, tag="v")
            nc.vector.tensor_copy(
                out=v_sb[:, :, :, D : D + 1],
                in_=onesf.rearrange("p (a b c) -> p a b c", a=KC, b=H_PER_CORE),
            )

            # -- kT(m, half) j-quarter: one [128,512] psum tile, 6 matmuls --
            # whole-width variant for the prologue (psA 4KB slots)
            def make_kt(m, half, pool, tag, evict_act=False):
                st8 = {}

                def part(clo, chi):
                    if "ps" not in st8:
                        st8["ps"] = pool.tile([128, 1024], f32, tag=tag,
                                              name=f"kps{m}{half}")
                    kps = st8["ps"]
                    for c in range(clo, chi):
                        for j in range(2):
                            nc.tensor.matmul(
                                kps[:, ts(j, 512)],
                                wk_sb[:, c, ts(m, 128)],
                                ctx_sb[:, c, ds(half * 1024 + j * 512, 512)],
                                start=(c == 0),
                                stop=(c == CK_CHUNKS - 1),
                            )

                def evict():
                    dst = kT_sb[:, m, ds(half * 1024, 1024)]
                    if evict_act:
                        nc.scalar.copy(out=dst, in_=st8["ps"])
                    else:
                        nc.vector.tensor_copy(out=dst, in_=st8["ps"])

                return part, evict

            # 512-key filler visit: computes kT_sb[:, m, q512*512 : +512]
            # in two <=3-matmul parts so no single slot gets a PE clump.
            def make_kt_q(m, q512, evict_act=False):
                st8 = {}

                def part(clo, chi, evict=False):
                    if "ps" not in st8:
                        st8["ps"] = psB.tile([128, 512], f32, tag="B", bufs=2,
                                             name=f"kq{m}{q512}")
                    kps = st8["ps"]
                    for c in range(clo, chi):
                        nc.tensor.matmul(
                            kps,
                            wk_sb[:, c, ts(m, 128)],
                            ctx_sb[:, c, ts(q512, 512)],
                            start=(c == 0),
                            stop=(c == CK_CHUNKS - 1),
                        )
                    if evict:
                        dst = kT_sb[:, m, ts(q512, 512)]
                        if evict_act:
                            nc.scalar.copy(out=dst, in_=kps)
                        else:
                            nc.vector.tensor_copy(out=dst, in_=kps)

                return (lambda: part(0, 3), lambda: part(3, 6, evict=True))

            # -- incremental qT(m) for a query block --
            def make_qt(qT_sb, x_sb, m, qb, pool, tag, evict_act=False):
                st8 = {}

                def part(klo, khi):
                    if "ps" not in st8:
                        st8["ps"] = pool.tile([128, QB], f32, tag=tag,
                                              name=f"qps{qb}{m}")
                    qps = st8["ps"]
                    for k in range(klo, khi):
                        for j in range(2):
                            nc.tensor.matmul(
                                qps[:, ts(j, 512)],
                                wq_sb[:, k, ts(m, 128)],
                                x_sb[:, k, ts(j, 512)],
                                start=(k == 0),
                                stop=(k == QK_CHUNKS - 1),
                            )

                def evict():
                    if evict_act:
                        nc.scalar.copy(out=qT_sb[:, m, :], in_=st8["ps"])
                    else:
                        nc.vector.tensor_copy(out=qT_sb[:, m, :], in_=st8["ps"])

                return part, evict

            def emit_v_chunk(kc, pool, tag, bufs=None):
                vps = pool.tile([128, IN_PER_CORE], f32, tag=tag, bufs=bufs,
                                name=f"vps{kc}")
                for c in range(CK_CHUNKS):
                    nc.tensor.matmul(
                        vps,
                        ctx_sb[:, c, ts(kc, 128)],
                        wv_sb[:, c, :],
                        start=(c == 0),
                        stop=(c == CK_CHUNKS - 1),
                    )
                nc.vector.tensor_copy(
                    out=v_sb[:, kc, :, 0:D],
                    in_=vps.rearrange("p (h d) -> p h d", h=H_PER_CORE),
                )

            # qT j-half filler: computes qT_sb[:, m, jh*512 : +512] in two
            # 4-matmul parts.
            def make_qt_j(qT_sb, x_sb, m, qb, jh):
                st8 = {}

                def part(klo, khi, evict=False):
                    if "ps" not in st8:
                        st8["ps"] = psB.tile([128, 512], f32, tag="B", bufs=2,
                                             name=f"qj{qb}{m}{jh}")
                    qps = st8["ps"]
                    for k in range(klo, khi):
                        nc.tensor.matmul(
                            qps,
                            wq_sb[:, k, ts(m, 128)],
                            x_sb[:, k, ts(jh, 512)],
                            start=(k == 0),
                            stop=(k == QK_CHUNKS - 1),
                        )
                    if evict:
                        nc.vector.tensor_copy(out=qT_sb[:, m, ts(jh, 512)],
                                              in_=qps)

                return (lambda: part(0, 4), lambda: part(4, 8, evict=True))

            # ---- prologue compute (runs in the DMA shadow) ----
            # kT(m0,K1) and kT(m1,K1) interleave per ctx-chunk arrival.
            kt00_part, kt00_evict = make_kt(0, 0, psA, "A")
            kt10_part, kt10_evict = make_kt(1, 0, psA, "A")
            for c in range(CK_CHUNKS):
                kt00_part(c, c + 1)
                kt10_part(c, c + 1)
            kt00_evict()
            kt10_evict()

            # v(kc0-7) and qT(m0,qb0) interleave: v paces with wv/ctx, qT
            # with the x0 chunks.
            qT0_sb = stream.tile([128, IN_CHUNKS, QB], f32r, tag="qt", bufs=1,
                                 name="qT0")
            qt00_part, qt00_evict = make_qt(qT0_sb, x0_sb, 0, 0, psA, "A",
                                            evict_act=True)
            for k in range(QK_CHUNKS):
                emit_v_chunk(k, psA, "A")
                qt00_part(k, k + 1)
            qt00_evict()

            qT1_sb = stream.tile([128, IN_CHUNKS, QB], f32r, tag="qt2", bufs=1,
                                 name="qT1")
            qT_tiles = [qT0_sb, qT1_sb]

            # ---- filler slot table ----
            # fill[(qb, h, kc)] -> list of closures emitted after exp(h, kc)
            fill = {}

            def add_fill(qb, h, kc, fn):
                fill.setdefault((qb, h, kc), []).append(fn)

            # Everything key-half-2 flavored is gated on the ctx-K2 DMAs,
            # which land ~30-37us (after x0); nothing K2-gated may be
            # emitted before h0-kc5 or it head-of-line blocks the PE queue.
            # kT(m0, K2) quarters: q2 gates S(h0, kc8) (Act reaches it
            # ~37.8us), q3 gates S(h0, kc12).  Evict on Act so it lands
            # right before exp(kc8)/exp(kc12) in the Act queue.
            # qT(m1, qb0) as h0's first fillers (kc0-3 carry nothing else);
            # gates only S(h2).  wq-m1 lands right after x0.
            qt01a, qt01b = make_qt_j(qT0_sb, x0_sb, 1, 0, 0)
            qt01c, qt01d = make_qt_j(qT0_sb, x0_sb, 1, 0, 1)
            add_fill(0, 0, 0, qt01a)
            add_fill(0, 0, 1, qt01b)
            add_fill(0, 0, 2, qt01c)
            add_fill(0, 0, 3, qt01d)

            ktq02a, ktq02b = make_kt_q(0, 2, evict_act=True)
            ktq03a, ktq03b = make_kt_q(0, 3, evict_act=True)
            add_fill(0, 0, 4, ktq02a)
            add_fill(0, 0, 5, ktq02b)
            add_fill(0, 0, 8, ktq03a)
            add_fill(0, 0, 9, ktq03b)

            # v kc8-15 singles (also K2-gated): PV(kc) is emitted at slot
            # kc+1 before that slot's fillers, so v(kc) sits at slot <= kc.
            for kc0, slot in ((8, 7), (9, 8), (10, 10), (11, 11), (12, 12),
                              (13, 13), (14, 14), (15, 15)):
                add_fill(0, 0, slot, lambda kc=kc0:
                         emit_v_chunk(kc, psB, "B", bufs=2))

            # qT(m1, qb0): gates S(h2).  wq-m1 lands ~39us.  Spread one
            # part per two slots so the PV stream never lags the pt ring.

            # kT(m1) 512-key quarters: q0/q1 gate S(h2, kc0/kc4); q2/q3
            # gate S(h2, kc8/kc12).  q3 spills into h2's early slots.
            # kT(m1, K2) quarters (K1 done in the prologue): gate
            # S(h2, kc8/kc12).
            ktq1 = [make_kt_q(1, q) for q in (2, 3)]
            add_fill(0, 1, 4, ktq1[0][0])
            add_fill(0, 1, 5, ktq1[0][1])
            add_fill(0, 1, 7, ktq1[1][0])
            add_fill(0, 1, 8, ktq1[1][1])

            # qT(qb1, m0): gates qb1-h0.  x1 chunks land as the x slot
            # frees (~48us + 1.5us/chunk -> all in by ~60us).
            qt10a, qt10b = make_qt_j(qT1_sb, x1_sb, 0, 1, 0)
            qt10c, qt10d = make_qt_j(qT1_sb, x1_sb, 0, 1, 1)
            add_fill(0, 2, 3, qt10a)
            add_fill(0, 2, 5, qt10b)
            add_fill(0, 2, 7, qt10c)
            add_fill(0, 2, 9, qt10d)

            # qT(qb1, m1): gates only qb1-h2, so it lives in qb1-h0/h1's
            # even slots (the odd ones carry qb0's final projections).
            qt11a, qt11b = make_qt_j(qT1_sb, x1_sb, 1, 1, 0)
            qt11c, qt11d = make_qt_j(qT1_sb, x1_sb, 1, 1, 1)
            add_fill(0, 3, 1, qt11a)
            add_fill(0, 3, 3, qt11b)
            add_fill(0, 3, 5, qt11c)
            add_fill(0, 3, 7, qt11d)

            # ---- final projection: one j-half of one 128-query chunk ----
            ost_tiles = {}

            def emit_final_half(qb, qm, jh, ot_all, evict_eng="dve", tag="B"):
                ops = psB.tile([128, 512], f32, tag=tag, bufs=2,
                               name=f"ops{qb}{qm}{jh}")
                for t in range(IN_CHUNKS):
                    nc.tensor.matmul(
                        ops,
                        ot_all[:, t, ts(qm, 128)],
                        wo_sb[:, t, ts(jh, 512)],
                        start=(t == 0),
                        stop=(t == IN_CHUNKS - 1),
                    )
                if (qb, qm) not in ost_tiles:
                    ost_tiles[(qb, qm)] = stream.tile(
                        [128, Q_DIM], f32, tag="ost", bufs=3, name=f"ost{qb}{qm}"
                    )
                ost = ost_tiles[(qb, qm)]
                if evict_eng == "act":
                    nc.scalar.copy(out=ost[:, ts(jh, 512)], in_=ops)
                else:
                    nc.vector.tensor_copy(out=ost[:, ts(jh, 512)], in_=ops)
                if jh == 1:
                    # SP (sync) HWDGE: SP is idle after the input loads
                    nc.sync.dma_start(
                        out=out_d[ds(qb * QB + qm * 128, 128), :], in_=ost
                    )

            # full-width final chunk on the psA ring -- for the tail, where
            # the score pipeline is done and psA is free.
            def emit_final_full(qb, qm, ot_all, evict_eng="dve"):
                ops = psA.tile([128, Q_DIM], f32, tag="A", name=f"opf{qb}{qm}")
                for t in range(IN_CHUNKS):
                    for j in range(2):
                        nc.tensor.matmul(
                            ops[:, ts(j, 512)],
                            ot_all[:, t, ts(qm, 128)],
                            wo_sb[:, t, ts(j, 512)],
                            start=(t == 0),
                            stop=(t == IN_CHUNKS - 1),
                        )
                ost = stream.tile([128, Q_DIM], f32, tag="ost", bufs=3,
                                  name=f"osf{qb}{qm}")
                if evict_eng == "act":
                    nc.scalar.copy(out=ost, in_=ops)
                else:
                    nc.vector.tensor_copy(out=ost, in_=ops)
                nc.sync.dma_start(
                    out=out_d[ds(qb * QB + qm * 128, 128), :], in_=ost
                )

            # ---- attention: one flat software-pipelined (qb, h, kc) stream --
            ot_alls = {}
            for qb in range(N_QB):
                ot_alls[qb] = stream.tile([128, IN_CHUNKS, QB], f32r,
                                          tag="otall", bufs=2, name=f"otall{qb}")
                # previous block's final chunk halves spread across this
                # block's h0-h2 loops.
                if qb > 0:
                    slots = ([(0, kc) for kc in range(1, 10, 2)]
                             + [(1, kc) for kc in range(1, 10, 2)]
                             + [(2, kc) for kc in range(1, 12, 2)])
                    for qm in range(QB // 128):
                        for jh in range(2):
                            h, kc = slots[qm * 2 + jh]
                            add_fill(qb, h, kc,
                                     lambda qb=qb, qm=qm, jh=jh:
                                     emit_final_half(qb - 1, qm, jh,
                                                     ot_alls[qb - 1]))

            def emit_pv(p):
                qb, h, kc, pt, ot_j = p
                for j in range(2):
                    nc.tensor.matmul(
                        ot_j[j],
                        v_sb[:, kc, h, :],
                        pt[:, ts(j, 512)],
                        start=(kc == 0),
                        stop=(kc == KC - 1),
                    )

            def emit_normalize(qb, h, ot_j, tail=False):
                t, po = h // 2, (h % 2) * D
                if tail:
                    # 256-wide pieces: shortest chain from last PV to the
                    # first final projection.
                    for s in range(4):
                        ot_raw = stream.tile([D + 1, 256], f32r, tag="otraw",
                                             bufs=2, name=f"otrz{qb}{h}{s}")
                        nc.scalar.copy(out=ot_raw, in_=ot_j[s // 2][:, ds((s % 2) * 256, 256)])
                        bc_ps = psB.tile([D, 256], f32, tag="B", bufs=2,
                                         name=f"bcz{qb}{h}{s}")
                        nc.tensor.matmul(bc_ps, ones_sb[D : D + 1, :],
                                         ot_raw[D : D + 1, :],
                                         start=True, stop=True)
                        nc.vector.reciprocal(out=bc_ps, in_=bc_ps)
                        nc.vector.tensor_mul(
                            out=ot_alls[qb][po : po + D, t, ds(s * 256, 256)],
                            in0=ot_raw[0:D, :],
                            in1=bc_ps,
                        )
                        for qm in range(s * 2, s * 2 + 2):
                            if qm % 2 == 0:
                                emit_final_full(qb, qm, ot_alls[qb],
                                                evict_eng="dve")
                            else:
                                hr = "B" if qm % 4 == 1 else "ot"
                                emit_final_half(qb, qm, 0, ot_alls[qb],
                                                evict_eng="act", tag=hr)
                                emit_final_half(qb, qm, 1, ot_alls[qb],
                                                evict_eng="act", tag=hr)
                    return
                for s in range(2):
                    ot_raw = stream.tile([D + 1, 512], f32r, tag="otraw",
                                         bufs=2, name=f"otraw{qb}{h}{s}")
                    if tail:
                        nc.scalar.copy(out=ot_raw, in_=ot_j[s])
                    else:
                        nc.vector.tensor_copy(out=ot_raw, in_=ot_j[s])
                    bc_ps = psB.tile([D, 512], f32, tag="B", bufs=2,
                                     name=f"bc{qb}{h}{s}")
                    nc.tensor.matmul(
                        bc_ps,
                        ones_sb[D : D + 1, :],
                        ot_raw[D : D + 1, :],
                        start=True,
                        stop=True,
                    )
                    nc.vector.reciprocal(out=bc_ps, in_=bc_ps)
                    nc.vector.tensor_mul(
                        out=ot_alls[qb][po : po + D, t, ds(s * 512, 512)],
                        in0=ot_raw[0:D, :],
                        in1=bc_ps,
                    )

            # Flat lead-2 stream: at step g the scores for step g+2 are
            # emitted first, so the Act queue always holds two ready exps
            # and fillers can never starve it; the st ring (2 bufs) then
            # throttles the PE to the exp chain's pace.
            steps = [(qb, h, kc)
                     for qb in range(N_QB)
                     for h in range(H_PER_CORE)
                     for kc in range(KC)]
            ot_js = {}  # (qb, h) -> [ot_j0, ot_j1]
            pts = {}    # step index -> pt tile

            def emit_S(g):
                qb, h, kc = steps[g]
                t, po = h // 2, (h % 2) * D
                st = psA.tile([128, QB], f32, tag="A", name=f"st{qb}{h}{kc}")
                for j in range(2):
                    nc.tensor.matmul(
                        st[:, ts(j, 512)],
                        kT_sb[po : po + D, t, ts(kc, 128)],
                        qT_tiles[qb][po : po + D, t, ts(j, 512)],
                        start=True,
                        stop=True,
                    )
                return st

            def emit_PV(g):
                qb, h, kc = steps[g]
                if (qb, h) not in ot_js:
                    ot_js[(qb, h)] = [
                        psB.tile([D + 1, 512], f32, tag="ot", bufs=2,
                                 name=f"ot{qb}{h}{j}")
                        for j in range(2)
                    ]
                emit_pv((qb, h, kc, pts.pop(g), ot_js[(qb, h)]))
                if kc == KC - 1:
                    emit_normalize(qb, h, ot_js.pop((qb, h)),
                                   tail=(g == len(steps) - 1))

            sts = {0: emit_S(0), 1: emit_S(1)}
            for g, (qb, h, kc) in enumerate(steps):
                if g + 2 < len(steps):
                    sts[g + 2] = emit_S(g + 2)
                if g >= 1:
                    emit_PV(g - 1)
                pt = stream.tile([128, QB], f32r, tag="pt", bufs=4,
                                 name=f"pt{qb}{h}{kc}")
                nc.scalar.activation(out=pt, in_=sts.pop(g), func=Exp,
                                     scale=SCALE)
                pts[g] = pt
                for fn in fill.pop((qb, h, kc), ()):
                    if fn is not None:
                        fn()

            # flush the very last step
            emit_PV(len(steps) - 1)

    nc.finalize()
    return nc


def _get_nc():
    global _CACHED_NC
    if _CACHED_NC is None:
        _CACHED_NC = _build_bass()
    return _CACHED_NC


def _numpy_fallback(x, context, mask, Wq, Wk, Wv, Wout, bout):
    q = (x @ Wq.T).reshape(B, N, H, D)
    k = (context @ Wk.T).reshape(B, M, H, D)
    v = (context @ Wv.T).reshape(B, M, H, D)
    sim = np.einsum("bnhd,bmhd->bhnm", q, k) * SCALE
    sim = np.where(mask[:, None, None, :], sim, -np.finfo(np.float32).max)
    sim -= sim.max(axis=-1, keepdims=True)
    attn = np.exp(sim)
    attn /= attn.sum(axis=-1, keepdims=True)
    out = np.einsum("bhnm,bmhd->bnhd", attn, v).reshape(B, N, INNER)
    return (out @ Wout.T + bout).astype(np.float32)


def kernel(x, context, mask, Wq, Wk, Wv, Wout, bout, _want_results=False):
    x = np.asarray(x, dtype=np.float32)
    context = np.asarray(context, dtype=np.float32)
    mask = np.asarray(mask)
    Wq = np.asarray(Wq, dtype=np.float32)
    Wk = np.asarray(Wk, dtype=np.float32)
    Wv = np.asarray(Wv, dtype=np.float32)
    Wout = np.asarray(Wout, dtype=np.float32)
    bout = np.asarray(bout, dtype=np.float32)

    if not mask.all():
        return _numpy_fallback(x, context, mask, Wq, Wk, Wv, Wout, bout)

    from concourse.bass_utils import run_bass_kernel_spmd

    in_maps = []
    for c in range(N_CORES):
        b, hg = c // 2, c % 2
        sl = slice(hg * IN_PER_CORE, (hg + 1) * IN_PER_CORE)
        in_maps.append(
            {
                "xT": np.ascontiguousarray(x[b].T),
                "cT": np.ascontiguousarray(context[b].T),
                "wq": np.ascontiguousarray(Wq[sl, :].T),
                "wk": np.ascontiguousarray(Wk[sl, :].T),
                "wv": np.ascontiguousarray(Wv[sl, :].T),
                "wo": np.ascontiguousarray(Wout[:, sl].T),
            }
        )

    res = run_bass_kernel_spmd(_get_nc(), in_maps, core_ids=list(range(N_CORES)))

    out = np.empty((B, N, Q_DIM), dtype=np.float32)
    for b in range(B):
        out[b] = res.results[2 * b]["out"] + res.results[2 * b + 1]["out"] + bout
    if _want_results:
        return out, res
    return out



# revision 5
# speedup vs baseline: 1.0767x; 1.0767x over previous
"""Trainium2 Bass kernel for CrossAttention (B=4, N=M=2048, H=8, D=64,
Q_DIM=1024, C_DIM=768).  v2: bf16 datapath + q-partitioned PV + split exp.

Sharding over 8 cores: core c handles batch b = c//2 and head-group
hg = c%2 (4 heads, 256 inner dims).  Each core computes a *partial*
output projection; the host sums core pairs and adds the output bias.

Key structure (chosen against the concourse TimelineSim cost model):
  - all matmul operands bf16 (1 cycle/row at any width); accumulation f32.
  - scores S.T[keys, q] per (qb, h, kc): 2 ap-512 matmuls (K=64).
  - PV is q-partitioned: out[128q, 65] per q-chunk with pt chunk stationary
    and v (with an appended ones-column -> softmax denominators) moving:
    520 cols per kc step instead of 1024 -> half the PE cost of the
    keys-partitioned form.  The 8 q-chunk accumulation groups share two
    PSUM banks via a single start/stop per bank (start marks the whole
    2KB zero region; first touch of each chunk overwrites).
  - exp is split: ~2/3 of score tiles on Act (activation Exp), ~1/3 via
    DVE copy to SBUF + GPSIMD pow(e^SCALE, S) (GPSIMD cannot read PSUM).
    Pool-path PV consumption is deferred 3 steps so the PE never waits.
  - normalization folds into the PV eviction: DVE reciprocal of the
    denominator column, then tensor_scalar_mul into packed bf16 tiles.
  - packed [128q, 128inner] head-pair tiles are PE-transposed (identity
    permutation rhs) so the output projection gets inner-contracted lhsT.
  - output projection per 128-query chunk: 2x2 ap-512 matmuls, evicted to
    SBUF and DMA'd per chunk.

The attention mask in this problem is all-True; if a mask with False
entries is ever passed, kernel() falls back to a numpy reference.
"""

import numpy as np

B, N, M = 4, 2048, 2048
Q_DIM, C_DIM, H, D = 1024, 768, 8, 64
INNER = H * D  # 512
SCALE = D ** -0.5

N_CORES = 8
H_PER_CORE = 4
IN_PER_CORE = H_PER_CORE * D  # 256
QB = 1024
N_QB = N // QB          # 2
KC = M // 128           # 16 key chunks
QK_CHUNKS = Q_DIM // 128   # 8
CK_CHUNKS = C_DIM // 128   # 6

# key-chunks whose exp runs on GPSIMD (via DVE psum->sbuf copy); the rest
# run on Act.  Deferred +3 steps before PV consumption.
POOL_KCS = (2, 5, 8, 11)
PV_LEAD_ACT = 1
PV_LEAD_POOL = 3

_CACHED_NC = None
_DEBUG = False


def _build_bass():
    import concourse.bass as bass
    import concourse.mybir as mybir
    import concourse.tile as tile
    from concourse import bacc

    f32 = mybir.dt.float32
    f32r = mybir.dt.float32r
    bf16 = mybir.dt.bfloat16
    ts, ds = bass.ts, bass.ds
    Exp = mybir.ActivationFunctionType.Exp
    Pow = mybir.AluOpType.pow

    nc = bacc.Bacc("TRN2", target_bir_lowering=False)

    # weights arrive pre-arranged in their SBUF layouts (one DMA each)
    xT = nc.dram_tensor("xT", [Q_DIM, N], bf16, kind="ExternalInput")
    cT = nc.dram_tensor("cT", [C_DIM, M], bf16, kind="ExternalInput")
    wq = nc.dram_tensor("wq", [128, QK_CHUNKS, IN_PER_CORE], bf16,
                        kind="ExternalInput")
    wk = nc.dram_tensor("wk", [128, CK_CHUNKS, IN_PER_CORE], bf16,
                        kind="ExternalInput")
    wv = nc.dram_tensor("wv", [128, CK_CHUNKS, IN_PER_CORE], bf16,
                        kind="ExternalInput")
    wo = nc.dram_tensor("wo", [128, 2, Q_DIM], bf16, kind="ExternalInput")
    ident_d = nc.dram_tensor("ident", [128, 128], bf16, kind="ExternalInput")
    out_d = nc.dram_tensor("out", [N, Q_DIM], f32, kind="ExternalOutput")
    if _DEBUG:
        dbg_kt = nc.dram_tensor("dbg_kt", [128, 2, M], bf16, kind="ExternalOutput")
        dbg_qt = nc.dram_tensor("dbg_qt", [128, 2, QB], bf16, kind="ExternalOutput")
        dbg_v = nc.dram_tensor("dbg_v", [128, KC, H_PER_CORE, D + 1], bf16, kind="ExternalOutput")
        dbg_ot = nc.dram_tensor("dbg_ot", [128, 2, QB], bf16, kind="ExternalOutput")
        dbg_pt = nc.dram_tensor("dbg_pt", [128, QB], bf16, kind="ExternalOutput")

    with tile.TileContext(nc) as tc:
        with (
            tc.tile_pool(name="persist", bufs=1) as persist,
            tc.tile_pool(name="stream", bufs=2) as stream,
            tc.tile_pool(name="psS", bufs=2, space="PSUM") as psS,
            tc.tile_pool(name="psPV", bufs=1, space="PSUM") as psPV,
            tc.tile_pool(name="psT", bufs=1, space="PSUM") as psT,
            tc.tile_pool(name="psO", bufs=1, space="PSUM") as psO,
        ):
            # ---- constants ----
            onesb = persist.tile([128, 64], bf16, tag="onesb")
            nc.vector.memset(onesb, 1.0)
            base = persist.tile([128, QB], f32, tag="base")
            nc.vector.memset(base, float(np.exp(SCALE)))
            wrm = persist.tile([128, 512], bf16, tag="wrm")
            nc.vector.memset(wrm, 0.0)
            # warm the Act exp table during the DMA shadow
            warm2 = persist.tile([128, 1], f32, tag="warm2")
            nc.scalar.activation(out=warm2, in_=wrm[:, 0:1], func=Exp,
                                 scale=SCALE)

            # PE p-state warmup: keep the tensor engine continuously busy
            # through the initial DMA wait so the first real matmuls run at
            # full clock (the cost model ramps 0.65->1.2->2.4 GHz over 3us
            # of continuous execution).
            warm_ps = psO.tile([128, 512], f32, tag="O", name="warmps")
            N_WARM = 13
            for i in range(N_WARM):
                nc.tensor.matmul(warm_ps, wrm[:, 0:128], wrm,
                                 start=(i == 0), stop=(i == N_WARM - 1))

            # ---- DMA schedule (execution order == emission order) ----
            # qT chain first (wq + x0-j0), then kT chain (wk + ctx-q0).
            ctx_sb = persist.tile([128, CK_CHUNKS, M], bf16, tag="ctx")
            cT_r = cT.rearrange("(c p) n -> p c n", p=128)

            def dma_ctx_quarter(q):
                nc.sync.dma_start(
                    out=ctx_sb[:, :, ds(q * 512, 512)],
                    in_=cT_r[:, :, ds(q * 512, 512)],
                )

            wq_sb = persist.tile([128, QK_CHUNKS, IN_PER_CORE], bf16, tag="wq")
            nc.sync.dma_start(out=wq_sb, in_=wq[:, :, :])

            # x0 split by j-half so qT(qb0, m0, j0) completes early and the
            # first score tile's j0 half can be exp'd before x0-j1 lands.
            xT_r = xT.rearrange("(k p) n -> p k n", p=128)
            x0_sb = stream.tile([128, QK_CHUNKS, QB], bf16, tag="x0", bufs=1,
                                name="x0")
            nc.sync.dma_start(out=x0_sb[:, :, 0:512], in_=xT_r[:, :, 0:512])

            wk_sb = persist.tile([128, CK_CHUNKS, IN_PER_CORE], bf16, tag="wk")
            nc.sync.dma_start(out=wk_sb, in_=wk[:, :, :])

            dma_ctx_quarter(0)

            wv_sb = persist.tile([128, CK_CHUNKS, IN_PER_CORE], bf16, tag="wv")
            nc.sync.dma_start(out=wv_sb, in_=wv[:, :, :])

            nc.sync.dma_start(out=x0_sb[:, :, 512:1024],
                              in_=xT_r[:, :, 512:1024])

            dma_ctx_quarter(1)
            dma_ctx_quarter(2)
            dma_ctx_quarter(3)

            x1_sb = stream.tile([128, QK_CHUNKS, QB], bf16, tag="x1", bufs=1,
                                name="x1")
            nc.sync.dma_start(out=x1_sb[:, 0:4, :],
                              in_=xT_r[:, 0:4, QB : 2 * QB])
            nc.sync.dma_start(out=x1_sb[:, 4:8, :],
                              in_=xT_r[:, 4:8, QB : 2 * QB])

            ident_sb = persist.tile([128, 128], bf16, tag="ident")
            nc.sync.dma_start(out=ident_sb, in_=ident_d[:, :])

            wo_sb = persist.tile([128, 2, Q_DIM], bf16, tag="wo")
            nc.sync.dma_start(out=wo_sb, in_=wo[:, :, :])

            # ---- persistent compute targets ----
            kT_sb = persist.tile([128, 2, M], bf16, tag="kt")
            v_sb = persist.tile([128, KC, H_PER_CORE, D + 1], bf16, tag="v")
            nc.vector.tensor_copy(
                out=v_sb[:, :, :, D : D + 1],
                in_=onesb.rearrange("p (a b c) -> p a b c", a=KC, b=H_PER_CORE),
            )

            # ---- projection pieces ----
            def emit_kt_q(m, q, pool, tag):
                """kT_sb[:, m, q*512:(q+1)*512] in one go (6 matmuls)."""
                kq = pool.tile([128, 512], f32, tag=tag, name=f"kq{m}{q}")
                for c in range(CK_CHUNKS):
                    nc.tensor.matmul(
                        kq,
                        wk_sb[:, c, ts(m, 128)],
                        ctx_sb[:, c, ts(q, 512)],
                        start=(c == 0),
                        stop=(c == CK_CHUNKS - 1),
                    )
                nc.vector.tensor_copy(out=kT_sb[:, m, ts(q, 512)], in_=kq)

            def emit_v_chunk(kc):
                # padded to 2KB so every psT tile shares one size class
                vps = psT.tile([128, 512], f32, tag="T", name=f"vps{kc}")
                for c in range(CK_CHUNKS):
                    nc.tensor.matmul(
                        vps[:, 0:IN_PER_CORE],
                        ctx_sb[:, c, ts(kc, 128)],
                        wv_sb[:, c, :],
                        start=(c == 0),
                        stop=(c == CK_CHUNKS - 1),
                    )
                nc.vector.tensor_copy(
                    out=v_sb[:, kc, :, 0:D],
                    in_=vps[:, 0:IN_PER_CORE].rearrange(
                        "p (h d) -> p h d", h=H_PER_CORE),
                )

            # atomic qT j-half (8 matmuls + evict; single-buffer pools need
            # each user to fully retire before the next opens)
            def emit_qt_j(qT_sb, x_sb, m, qb, jh, pool=None, tag=None):
                pool = pool or psO
                tag = tag or "O"
                qps = pool.tile([128, 512], f32, tag=tag,
                                name=f"qj{qb}{m}{jh}")
                for k in range(QK_CHUNKS):
                    nc.tensor.matmul(
                        qps,
                        wq_sb[:, k, ts(m, 128)],
                        x_sb[:, k, ts(jh, 512)],
                        start=(k == 0),
                        stop=(k == QK_CHUNKS - 1),
                    )
                nc.vector.tensor_copy(out=qT_sb[:, m, ts(jh, 512)], in_=qps)

            # ---- prologue (PE order matches DMA arrival order) ----
            qT0_sb = stream.tile([128, 2, QB], bf16, tag="qt0", bufs=1,
                                 name="qT0")
            emit_qt_j(qT0_sb, x0_sb, 0, 0, 0, pool=psS, tag="S")
            emit_kt_q(0, 0, psS, "S")
            emit_v_chunk(0)
            emit_v_chunk(1)
            emit_qt_j(qT0_sb, x0_sb, 0, 0, 1, pool=psS, tag="S")
            emit_v_chunk(2)
            emit_v_chunk(3)

            qT1_sb = stream.tile([128, 2, QB], bf16, tag="qt1", bufs=1,
                                 name="qT1")
            qT_tiles = [qT0_sb, qT1_sb]

            # ---- filler slot table: fill[(qb, h, kc)] -> list of fns ----
            fill = {}

            def add_fill(qb, h, kc, fn):
                fill.setdefault((qb, h, kc), []).append(fn)

            # kT(m0, q1..q3) gate S(h0, kc4/8/12).  Each filler slot is
            # placed near the step where its gating ctx quarter lands so PE
            # head-of-line stalls stay short, but strictly before the lead-2
            # S emission that reads the result.
            add_fill(0, 0, 1, lambda: emit_kt_q(0, 1, psO, "O"))
            add_fill(0, 0, 5, lambda: emit_kt_q(0, 2, psO, "O"))
            add_fill(0, 0, 9, lambda: emit_kt_q(0, 3, psO, "O"))
            # v(kc4..15) gate PV(h0, kc) due at step kc+1 (act) / kc+3 (pool)
            for kc0, slot in ((4, 2), (5, 3), (6, 4), (7, 5), (8, 6), (9, 7),
                              (10, 8), (11, 10), (12, 11), (13, 12), (14, 13),
                              (15, 14)):
                add_fill(0, 0, slot, lambda kc=kc0: emit_v_chunk(kc))
            # kT(m1, q0..q3) gate S(h2, *): fill h1 slots (ctx all landed).
            add_fill(0, 1, 1, lambda: emit_kt_q(1, 0, psO, "O"))
            add_fill(0, 1, 3, lambda: emit_kt_q(1, 1, psO, "O"))
            add_fill(0, 1, 5, lambda: emit_kt_q(1, 2, psO, "O"))
            add_fill(0, 1, 7, lambda: emit_kt_q(1, 3, psO, "O"))
            # qT(qb0, m1): gates S(h2) (emitted step 30); wq-m1 lands ~26us.
            add_fill(0, 1, 9, lambda: emit_qt_j(qT0_sb, x0_sb, 1, 0, 0))
            add_fill(0, 1, 11, lambda: emit_qt_j(qT0_sb, x0_sb, 1, 0, 1))
            # qT(qb1, m0): gates S(qb1, h0) (emitted step 62); x1 by ~35us.
            add_fill(0, 2, 1, lambda: emit_qt_j(qT1_sb, x1_sb, 0, 1, 0))
            add_fill(0, 2, 3, lambda: emit_qt_j(qT1_sb, x1_sb, 0, 1, 1))
            # qT(qb1, m1): gates S(qb1, h2) (emitted step 94).
            add_fill(0, 2, 5, lambda: emit_qt_j(qT1_sb, x1_sb, 1, 1, 0))
            add_fill(0, 2, 7, lambda: emit_qt_j(qT1_sb, x1_sb, 1, 1, 1))

            # ---- final projection: one (qm, jh) half ----
            ost_tiles = {}
            opool_toggle = {}

            def emit_final_half(qb, qm, jh, pool=None, tag=None,
                                evict_act=False):
                pool = pool or psO
                tag = tag or "O"
                ops = pool.tile([128, 512], f32, tag=tag,
                                name=f"ops{qb}{qm}{jh}")
                for t in range(2):
                    nc.tensor.matmul(
                        ops,
                        ot_alls[qb][:, t, ts(qm, 128)],
                        wo_sb[:, t, ts(jh, 512)],
                        start=(t == 0),
                        stop=(t == 1),
                    )
                if (qb, qm) not in ost_tiles:
                    ost_tiles[(qb, qm)] = stream.tile(
                        [128, Q_DIM], f32, tag="ost", bufs=3,
                        name=f"ost{qb}{qm}"
                    )
                ost = ost_tiles[(qb, qm)]
                if evict_act:
                    nc.scalar.copy(out=ost[:, ts(jh, 512)], in_=ops)
                else:
                    nc.vector.tensor_copy(out=ost[:, ts(jh, 512)], in_=ops)
                if jh == 1:
                    nc.sync.dma_start(
                        out=out_d[ds(qb * QB + qm * 128, 128), :], in_=ost
                    )

            # ---- attention state ----
            ot_alls = {}
            pv_banks = {}    # (qb, h) -> [tile_a, tile_b]
            pk_tiles = {}    # (qb, pair) -> packed normalized tile
            pts = {}         # step -> pt tile
            sts = {}         # step -> score tile (act) or sbuf copy (pool)

            steps = [(qb, h, kc)
                     for qb in range(N_QB)
                     for h in range(H_PER_CORE)
                     for kc in range(KC)]
            NSTEP = len(steps)

            # per-step PV due lists: pv_due[g] = list of source steps
            pv_due = [[] for _ in range(NSTEP + PV_LEAD_POOL + 1)]
            for g, (qb, h, kc) in enumerate(steps):
                lead = PV_LEAD_POOL if kc in POOL_KCS else PV_LEAD_ACT
                pv_due[g + lead].append(g)
            # per-(qb,h) bank emission bookkeeping: which (kc, j) is first/
            # last per bank in emission order.
            bank_seq = {}  # (qb, h, bank) -> [positions...] as (kc, j)
            for g, (qb, h, kc) in enumerate(steps):
                lead = PV_LEAD_POOL if kc in POOL_KCS else PV_LEAD_ACT
                for j in range(8):
                    bank_seq.setdefault((qb, h, j // 4), []).append(
                        (g + lead, kc, j))
            for key in bank_seq:
                bank_seq[key].sort()
            bank_first = {k: v[0] for k, v in bank_seq.items()}
            bank_last = {k: v[-1] for k, v in bank_seq.items()}

            def emit_S(g):
                qb, h, kc = steps[g]
                t, po = h // 2, (h % 2) * D
                st = psS.tile([128, QB], f32, tag="S", name=f"st{qb}{h}{kc}")
                for j in range(2):
                    nc.tensor.matmul(
                        st[:, ts(j, 512)],
                        kT_sb[po : po + D, t, ts(kc, 128)],
                        qT_tiles[qb][po : po + D, t, ts(j, 512)],
                        start=True,
                        stop=True,
                    )
                return st

            dbg_pt_tile = {}

            def emit_exp(g):
                qb, h, kc = steps[g]
                pt = stream.tile([128, QB], bf16, tag="pt", bufs=5,
                                 name=f"pt{qb}{h}{kc}")
                if _DEBUG and g == 0:
                    dbg_pt_tile["t"] = stream.tile([128, QB], bf16, tag="dbgpt",
                                                   bufs=1, name="dbgpt")
                st = sts.pop(g)
                if kc in POOL_KCS:
                    st_sb = stream.tile([128, QB], f32, tag="stsb", bufs=2,
                                        name=f"stsb{qb}{h}{kc}")
                    nc.vector.tensor_copy(out=st_sb, in_=st)
                    nc.gpsimd.tensor_tensor(out=pt, in0=base, in1=st_sb,
                                            op=Pow)
                elif g == 0:
                    # split halves: exp(j0) can run before x0-j1 has landed
                    for j in range(2):
                        nc.scalar.activation(out=pt[:, ts(j, 512)],
                                             in_=st[:, ts(j, 512)],
                                             func=Exp, scale=SCALE)
                else:
                    nc.scalar.activation(out=pt, in_=st, func=Exp,
                                         scale=SCALE)
                if _DEBUG and g == 0:
                    nc.vector.tensor_copy(out=dbg_pt_tile["t"], in_=pt)
                pts[g] = pt

            def emit_PV(src_g):
                qb, h, kc = steps[src_g]
                if (qb, h) not in pv_banks:
                    pv_banks[(qb, h)] = [
                        psPV.tile([128, 4, D + 1], f32, tag=f"pv{bk}",
                                  name=f"pv{qb}{h}{bk}")
                        for bk in range(2)
                    ]
                pt = pts.pop(src_g)
                banks = pv_banks[(qb, h)]
                for j in range(8):
                    bk = j // 4
                    nc.tensor.matmul(
                        banks[bk][:, j % 4, :],
                        pt[:, ts(j, 128)],
                        v_sb[:, kc, h, :],
                        start=((kc, j) == bank_first[(qb, h, bk)][1:]),
                        stop=((kc, j) == bank_last[(qb, h, bk)][1:]),
                    )

            def emit_norm(qb, h):
                """recip + normalize-evict into the packed pair tile."""
                banks = pv_banks.pop((qb, h))
                pair, slot = h // 2, h % 2
                if (qb, pair) not in pk_tiles:
                    pk_tiles[(qb, pair)] = stream.tile(
                        [128, 8, 2, D], bf16, tag="pk", bufs=2,
                        name=f"pk{qb}{pair}"
                    )
                pk = pk_tiles[(qb, pair)]
                recip = stream.tile([128, 8], f32, tag="recip", bufs=2,
                                    name=f"rc{qb}{h}")
                for bk in range(2):
                    nc.vector.reciprocal(out=recip[:, ds(bk * 4, 4)],
                                         in_=banks[bk][:, :, D])
                for j in range(8):
                    nc.vector.tensor_scalar_mul(
                        pk[:, j, slot, :],
                        banks[j // 4][:, j % 4, 0:D],
                        recip[:, ds(j, 1)],
                    )

            def emit_transpose(qb, pair):
                pk = pk_tiles.pop((qb, pair))
                trp = psT.tile([128, 8, 128], bf16, tag="T",
                               name=f"trp{qb}{pair}")
                for qc in range(8):
                    nc.tensor.matmul(
                        trp[:, qc, :],
                        pk[:, qc, :, :],
                        ident_sb,
                        start=True,
                        stop=True,
                        is_transpose=True,
                    )
                if qb not in ot_alls:
                    ot_alls[qb] = stream.tile([128, 2, QB], bf16,
                                              tag="otall", bufs=2,
                                              name=f"otall{qb}")
                nc.vector.tensor_copy(
                    out=ot_alls[qb][:, pair, :],
                    in_=trp.rearrange("p a b -> p (a b)"),
                )

            # outproj(qb0) in qb1's POOL-step slots: Act idles there, so the
            # eviction rides free on Act and DVE stays clear.
            for qm in range(8):
                for jh in range(2):
                    slot = qm * 2 + jh  # 0..15
                    add_fill(1, slot // 4, POOL_KCS[slot % 4],
                             lambda qm=qm, jh=jh: emit_final_half(
                                 0, qm, jh, evict_act=True))

            # ---- flat software-pipelined stream ----
            sts[0] = emit_S(0)
            sts[1] = emit_S(1)
            for g, (qb, h, kc) in enumerate(steps):
                if g + 2 < NSTEP:
                    sts[g + 2] = emit_S(g + 2)
                emit_exp(g)
                for src in pv_due[g]:
                    emit_PV(src)
                # head completion: when the last PV emission position for
                # (qb', h') was at this g, normalize (and transpose on pairs)
                for key, (lg, lkc, lj) in list(bank_last.items()):
                    pqb, ph, pbk = key
                    if pbk == 1 and lg == g:
                        emit_norm(pqb, ph)
                        if ph % 2 == 1:
                            emit_transpose(pqb, ph // 2)
                for fn in fill.pop((qb, h, kc), ()):
                    fn()

            # ---- drain ----
            for g in range(NSTEP, NSTEP + PV_LEAD_POOL + 1):
                for src in pv_due[g]:
                    emit_PV(src)
                for key, (lg, lkc, lj) in list(bank_last.items()):
                    pqb, ph, pbk = key
                    if pbk == 1 and lg == g:
                        emit_norm(pqb, ph)
                        if ph % 2 == 1:
                            emit_transpose(pqb, ph // 2)

            if _DEBUG:
                nc.sync.dma_start(out=dbg_kt[:, :, :], in_=kT_sb)
                nc.sync.dma_start(out=dbg_qt[:, :, :], in_=qT0_sb)
                nc.sync.dma_start(out=dbg_v[:, :, :, :], in_=v_sb)
                nc.sync.dma_start(out=dbg_ot[:, :, :], in_=ot_alls[0])
                nc.sync.dma_start(out=dbg_pt[:, :], in_=dbg_pt_tile["t"])

            # tail: qb1 output projection, alternating psO/psT banks and
            # DVE/Act evictions so the two chains pipeline
            for qm in range(8):
                for jh in range(2):
                    if (qm + jh) % 2 == 0:
                        emit_final_half(1, qm, jh, evict_act=False)
                    else:
                        emit_final_half(1, qm, jh, pool=psT, tag="T",
                                        evict_act=True)

    nc.finalize()
    return nc


def _get_nc():
    global _CACHED_NC
    if _CACHED_NC is None:
        _CACHED_NC = _build_bass()
    return _CACHED_NC


def _numpy_fallback(x, context, mask, Wq, Wk, Wv, Wout, bout):
    q = (x @ Wq.T).reshape(B, N, H, D)
    k = (context @ Wk.T).reshape(B, M, H, D)
    v = (context @ Wv.T).reshape(B, M, H, D)
    sim = np.einsum("bnhd,bmhd->bhnm", q, k) * SCALE
    sim = np.where(mask[:, None, None, :], sim, -np.finfo(np.float32).max)
    sim -= sim.max(axis=-1, keepdims=True)
    attn = np.exp(sim)
    attn /= attn.sum(axis=-1, keepdims=True)
    out = np.einsum("bhnm,bmhd->bnhd", attn, v).reshape(B, N, INNER)
    return (out @ Wout.T + bout).astype(np.float32)


def kernel(x, context, mask, Wq, Wk, Wv, Wout, bout, _want_results=False):
    import ml_dtypes

    bf = ml_dtypes.bfloat16
    x = np.asarray(x, dtype=np.float32)
    context = np.asarray(context, dtype=np.float32)
    mask = np.asarray(mask)
    Wq = np.asarray(Wq, dtype=np.float32)
    Wk = np.asarray(Wk, dtype=np.float32)
    Wv = np.asarray(Wv, dtype=np.float32)
    Wout = np.asarray(Wout, dtype=np.float32)
    bout = np.asarray(bout, dtype=np.float32)

    if not mask.all():
        return _numpy_fallback(x, context, mask, Wq, Wk, Wv, Wout, bout)

    from concourse.bass_utils import run_bass_kernel_spmd

    ident = np.eye(128, dtype=np.float32).astype(bf)
    in_maps = []
    for c in range(N_CORES):
        b, hg = c // 2, c % 2
        sl = slice(hg * IN_PER_CORE, (hg + 1) * IN_PER_CORE)
        in_maps.append(
            {
                "xT": np.ascontiguousarray(x[b].T).astype(bf),
                "cT": np.ascontiguousarray(context[b].T).astype(bf),
                "wq": np.ascontiguousarray(
                    Wq[sl, :].T.reshape(8, 128, 256).transpose(1, 0, 2)
                ).astype(bf),
                "wk": np.ascontiguousarray(
                    Wk[sl, :].T.reshape(6, 128, 256).transpose(1, 0, 2)
                ).astype(bf),
                "wv": np.ascontiguousarray(
                    Wv[sl, :].T.reshape(6, 128, 256).transpose(1, 0, 2)
                ).astype(bf),
                "wo": np.ascontiguousarray(
                    Wout[:, sl].T.reshape(2, 128, 1024).transpose(1, 0, 2)
                ).astype(bf),
                "ident": ident,
            }
        )

    res = run_bass_kernel_spmd(_get_nc(), in_maps, core_ids=list(range(N_CORES)))

    out = np.empty((B, N, Q_DIM), dtype=np.float32)
    for b in range(B):
        out[b] = res.results[2 * b]["out"] + res.results[2 * b + 1]["out"] + bout
    if _want_results:
        return out, res
    return out


# revision 7
# speedup vs baseline: 1.1405x; 1.0593x over previous
"""Trainium2 Bass kernel for CrossAttention (B=4, N=M=2048, H=8, D=64,
Q_DIM=1024, C_DIM=768).  v2: bf16 datapath + q-partitioned PV + split exp.

Sharding over 8 cores: core c handles batch b = c//2 and head-group
hg = c%2 (4 heads, 256 inner dims).  Each core computes a *partial*
output projection; the host sums core pairs and adds the output bias.

Key structure (chosen against the concourse TimelineSim cost model):
  - all matmul operands bf16 (1 cycle/row at any width); accumulation f32.
  - scores S.T[keys, q] per (qb, h, kc): 2 ap-512 matmuls (K=64).
  - PV is q-partitioned: out[128q, 65] per q-chunk with pt chunk stationary
    and v (with an appended ones-column -> softmax denominators) moving:
    520 cols per kc step instead of 1024 -> half the PE cost of the
    keys-partitioned form.  The 8 q-chunk accumulation groups share two
    PSUM banks via a single start/stop per bank (start marks the whole
    2KB zero region; first touch of each chunk overwrites).
  - exp is split: ~2/3 of score tiles on Act (activation Exp), ~1/3 via
    DVE copy to SBUF + GPSIMD pow(e^SCALE, S) (GPSIMD cannot read PSUM).
    Pool-path PV consumption is deferred 3 steps so the PE never waits.
  - normalization folds into the PV eviction: DVE reciprocal of the
    denominator column, then tensor_scalar_mul into packed bf16 tiles.
  - packed [128q, 128inner] head-pair tiles are PE-transposed (identity
    permutation rhs) so the output projection gets inner-contracted lhsT.
  - output projection per 128-query chunk: 2x2 ap-512 matmuls, evicted to
    SBUF and DMA'd per chunk.

The attention mask in this problem is all-True; if a mask with False
entries is ever passed, kernel() falls back to a numpy reference.
"""

import numpy as np

B, N, M = 4, 2048, 2048
Q_DIM, C_DIM, H, D = 1024, 768, 8, 64
INNER = H * D  # 512
SCALE = D ** -0.5

N_CORES = 8
H_PER_CORE = 4
IN_PER_CORE = H_PER_CORE * D  # 256
QB = 1024
N_QB = N // QB          # 2
KC = M // 128           # 16 key chunks
QK_CHUNKS = Q_DIM // 128   # 8
CK_CHUNKS = C_DIM // 128   # 6

# key-chunks whose exp runs on GPSIMD (via DVE psum->sbuf copy); the rest
# run on Act.  Pool-path score tiles live OUTSIDE the 2-deep psS ring (one
# 512 half each in psT/psO) so the Act exp chain never waits on the copy;
# their PV consumption is deferred 4 steps.  The two DMA-paced early heads
# stay all-Act.
POOL_KCS = (3, 6, 9, 11)
NO_POOL_HEADS = ((0, 0), (0, 1))
PV_LEAD_ACT = 1
PV_LEAD_POOL = 4

_CACHED_NC = None
_DEBUG = False


def _build_bass():
    import concourse.bass as bass
    import concourse.mybir as mybir
    import concourse.tile as tile
    from concourse import bacc

    f32 = mybir.dt.float32
    f32r = mybir.dt.float32r
    bf16 = mybir.dt.bfloat16
    ts, ds = bass.ts, bass.ds
    Exp = mybir.ActivationFunctionType.Exp
    Pow = mybir.AluOpType.pow

    nc = bacc.Bacc("TRN2", target_bir_lowering=False)

    # weights arrive pre-arranged in their SBUF layouts (one DMA each)
    xT = nc.dram_tensor("xT", [Q_DIM, N], bf16, kind="ExternalInput")
    cT = nc.dram_tensor("cT", [C_DIM, M], bf16, kind="ExternalInput")
    wq = nc.dram_tensor("wq", [128, QK_CHUNKS, IN_PER_CORE], bf16,
                        kind="ExternalInput")
    wk = nc.dram_tensor("wk", [128, CK_CHUNKS, IN_PER_CORE], bf16,
                        kind="ExternalInput")
    wv = nc.dram_tensor("wv", [128, CK_CHUNKS, IN_PER_CORE], bf16,
                        kind="ExternalInput")
    wo = nc.dram_tensor("wo", [128, 2, Q_DIM], bf16, kind="ExternalInput")
    ident_d = nc.dram_tensor("ident", [128, 128], bf16, kind="ExternalInput")
    out_d = nc.dram_tensor("out", [N, Q_DIM], bf16, kind="ExternalOutput")
    if _DEBUG:
        dbg_kt = nc.dram_tensor("dbg_kt", [128, 2, M], bf16, kind="ExternalOutput")
        dbg_qt = nc.dram_tensor("dbg_qt", [128, 2, QB], bf16, kind="ExternalOutput")
        dbg_v = nc.dram_tensor("dbg_v", [128, KC, H_PER_CORE, D + 1], bf16, kind="ExternalOutput")
        dbg_ot = nc.dram_tensor("dbg_ot", [128, 2, QB], bf16, kind="ExternalOutput")
        dbg_pt = nc.dram_tensor("dbg_pt", [128, QB], bf16, kind="ExternalOutput")

    with tile.TileContext(nc) as tc:
        with (
            tc.tile_pool(name="persist", bufs=1) as persist,
            tc.tile_pool(name="stream", bufs=2) as stream,
            tc.tile_pool(name="psS", bufs=2, space="PSUM") as psS,
            tc.tile_pool(name="psPV", bufs=1, space="PSUM") as psPV,
            tc.tile_pool(name="psT", bufs=1, space="PSUM") as psT,
            tc.tile_pool(name="psO", bufs=1, space="PSUM") as psO,
        ):
            # ---- constants ----
            onesb = persist.tile([128, 64], bf16, tag="onesb")
            nc.vector.memset(onesb, 1.0)
            base = persist.tile([128, QB], f32, tag="base")
            nc.vector.memset(base, float(np.exp(SCALE)))
            wrm = persist.tile([128, 512], bf16, tag="wrm")
            nc.vector.memset(wrm, 0.0)
            # warm the Act exp table during the DMA shadow
            warm2 = persist.tile([128, 1], f32, tag="warm2")
            nc.scalar.activation(out=warm2, in_=wrm[:, 0:1], func=Exp,
                                 scale=SCALE)

            # PE p-state warmup: keep the tensor engine continuously busy
            # through the initial DMA wait so the first real matmuls run at
            # full clock (the cost model ramps 0.65->1.2->2.4 GHz over 3us
            # of continuous execution).
            warm_ps = psO.tile([128, 512], f32, tag="O", name="warmps")
            N_WARM = 13
            for i in range(N_WARM):
                nc.tensor.matmul(warm_ps, wrm[:, 0:128], wrm,
                                 start=(i == 0), stop=(i == N_WARM - 1))

            # ---- DMA schedule (execution order == emission order) ----
            # qT chain first (wq + x0-j0), then kT chain (wk + ctx-q0).
            ctx_sb = persist.tile([128, CK_CHUNKS, M], bf16, tag="ctx")
            cT_r = cT.rearrange("(c p) n -> p c n", p=128)

            def dma_ctx_quarter(q):
                nc.sync.dma_start(
                    out=ctx_sb[:, :, ds(q * 512, 512)],
                    in_=cT_r[:, :, ds(q * 512, 512)],
                )

            wq_sb = persist.tile([128, QK_CHUNKS, IN_PER_CORE], bf16, tag="wq")
            nc.sync.dma_start(out=wq_sb, in_=wq[:, :, :])

            # x0 split by j-half so qT(qb0, m0, j0) completes early and the
            # first score tile's j0 half can be exp'd before x0-j1 lands.
            xT_r = xT.rearrange("(k p) n -> p k n", p=128)
            x0_sb = stream.tile([128, QK_CHUNKS, QB], bf16, tag="x0", bufs=1,
                                name="x0")
            nc.sync.dma_start(out=x0_sb[:, :, 0:512], in_=xT_r[:, :, 0:512])

            wk_sb = persist.tile([128, CK_CHUNKS, IN_PER_CORE], bf16, tag="wk")
            nc.sync.dma_start(out=wk_sb, in_=wk[:, :, :])

            dma_ctx_quarter(0)

            nc.sync.dma_start(out=x0_sb[:, :, 512:1024],
                              in_=xT_r[:, :, 512:1024])

            wv_sb = persist.tile([128, CK_CHUNKS, IN_PER_CORE], bf16, tag="wv")
            nc.sync.dma_start(out=wv_sb, in_=wv[:, :, :])

            dma_ctx_quarter(1)
            dma_ctx_quarter(2)
            dma_ctx_quarter(3)

            x1_sb = stream.tile([128, QK_CHUNKS, QB], bf16, tag="x1", bufs=1,
                                name="x1")
            nc.sync.dma_start(out=x1_sb[:, 0:4, :],
                              in_=xT_r[:, 0:4, QB : 2 * QB])
            nc.sync.dma_start(out=x1_sb[:, 4:8, :],
                              in_=xT_r[:, 4:8, QB : 2 * QB])

            ident_sb = persist.tile([128, 128], bf16, tag="ident")
            nc.sync.dma_start(out=ident_sb, in_=ident_d[:, :])

            wo_sb = persist.tile([128, 2, Q_DIM], bf16, tag="wo")
            nc.sync.dma_start(out=wo_sb, in_=wo[:, :, :])

            # ---- persistent compute targets ----
            kT_sb = persist.tile([128, 2, M], bf16, tag="kt")
            v_sb = persist.tile([128, KC, H_PER_CORE, D + 1], bf16, tag="v")
            nc.vector.tensor_copy(
                out=v_sb[:, :, :, D : D + 1],
                in_=onesb.rearrange("p (a b c) -> p a b c", a=KC, b=H_PER_CORE),
            )

            # ---- projection pieces ----
            def emit_kt_q(m, q, pool, tag):
                """kT_sb[:, m, q*512:(q+1)*512] in one go (6 matmuls)."""
                kq = pool.tile([128, 512], f32, tag=tag, name=f"kq{m}{q}")
                for c in range(CK_CHUNKS):
                    nc.tensor.matmul(
                        kq,
                        wk_sb[:, c, ts(m, 128)],
                        ctx_sb[:, c, ts(q, 512)],
                        start=(c == 0),
                        stop=(c == CK_CHUNKS - 1),
                    )
                nc.vector.tensor_copy(out=kT_sb[:, m, ts(q, 512)], in_=kq)

            def emit_v_chunk(kc):
                # padded to 2KB so every psT tile shares one size class
                vps = psT.tile([128, 512], f32, tag="T", name=f"vps{kc}")
                for c in range(CK_CHUNKS):
                    nc.tensor.matmul(
                        vps[:, 0:IN_PER_CORE],
                        ctx_sb[:, c, ts(kc, 128)],
                        wv_sb[:, c, :],
                        start=(c == 0),
                        stop=(c == CK_CHUNKS - 1),
                    )
                nc.vector.tensor_copy(
                    out=v_sb[:, kc, :, 0:D],
                    in_=vps[:, 0:IN_PER_CORE].rearrange(
                        "p (h d) -> p h d", h=H_PER_CORE),
                )

            # atomic qT j-half (8 matmuls + evict; single-buffer pools need
            # each user to fully retire before the next opens)
            def emit_qt_j(qT_sb, x_sb, m, qb, jh, pool=None, tag=None):
                pool = pool or psO
                tag = tag or "O"
                qps = pool.tile([128, 512], f32, tag=tag,
                                name=f"qj{qb}{m}{jh}")
                for k in range(QK_CHUNKS):
                    nc.tensor.matmul(
                        qps,
                        wq_sb[:, k, ts(m, 128)],
                        x_sb[:, k, ts(jh, 512)],
                        start=(k == 0),
                        stop=(k == QK_CHUNKS - 1),
                    )
                nc.vector.tensor_copy(out=qT_sb[:, m, ts(jh, 512)], in_=qps)

            # ---- prologue (PE order matches DMA arrival order) ----
            qT0_sb = stream.tile([128, 2, QB], bf16, tag="qt0", bufs=1,
                                 name="qT0")
            emit_qt_j(qT0_sb, x0_sb, 0, 0, 0, pool=psS, tag="S")
            emit_kt_q(0, 0, psS, "S")
            emit_v_chunk(0)
            emit_v_chunk(1)
            emit_qt_j(qT0_sb, x0_sb, 0, 0, 1, pool=psS, tag="S")
            emit_v_chunk(2)
            emit_v_chunk(3)

            qT1_sb = stream.tile([128, 2, QB], bf16, tag="qt1", bufs=1,
                                 name="qT1")
            qT_tiles = [qT0_sb, qT1_sb]

            # ---- filler slot table: fill[(qb, h, kc)] -> list of fns ----
            fill = {}

            def add_fill(qb, h, kc, fn):
                fill.setdefault((qb, h, kc), []).append(fn)

            # kT(m0, q1..q3) gate S(h0, kc4/8/12).  Each filler slot is
            # placed near the step where its gating ctx quarter lands so PE
            # head-of-line stalls stay short, but strictly before the lead-2
            # S emission that reads the result.
            add_fill(0, 0, 1, lambda: emit_kt_q(0, 1, psO, "O"))
            add_fill(0, 0, 5, lambda: emit_kt_q(0, 2, psO, "O"))
            add_fill(0, 0, 9, lambda: emit_kt_q(0, 3, psO, "O"))
            # v(kc4..15) gate PV(h0, kc) due at step kc+1 (act) / kc+3 (pool)
            for kc0, slot in ((4, 2), (5, 3), (6, 4), (7, 5), (8, 6), (9, 7),
                              (10, 8), (11, 10), (12, 11), (13, 12), (14, 13),
                              (15, 14)):
                add_fill(0, 0, slot, lambda kc=kc0: emit_v_chunk(kc))
            # kT(m1, q0..q3) gate S(h2, *): fill h1 slots (ctx all landed).
            add_fill(0, 1, 1, lambda: emit_kt_q(1, 0, psO, "O"))
            add_fill(0, 1, 3, lambda: emit_kt_q(1, 1, psO, "O"))
            add_fill(0, 1, 5, lambda: emit_kt_q(1, 2, psO, "O"))
            add_fill(0, 1, 7, lambda: emit_kt_q(1, 3, psO, "O"))
            # qT(qb0, m1): gates S(h2) (emitted step 30); wq-m1 lands ~26us.
            add_fill(0, 1, 9, lambda: emit_qt_j(qT0_sb, x0_sb, 1, 0, 0))
            add_fill(0, 1, 11, lambda: emit_qt_j(qT0_sb, x0_sb, 1, 0, 1))
            # qT(qb1, m0): gates S(qb1, h0) (emitted step 62); x1 by ~35us.
            add_fill(0, 2, 1, lambda: emit_qt_j(qT1_sb, x1_sb, 0, 1, 0))
            add_fill(0, 2, 3, lambda: emit_qt_j(qT1_sb, x1_sb, 0, 1, 1))
            # qT(qb1, m1): gates S(qb1, h2) (emitted step 94).
            add_fill(0, 2, 5, lambda: emit_qt_j(qT1_sb, x1_sb, 1, 1, 0))
            add_fill(0, 2, 7, lambda: emit_qt_j(qT1_sb, x1_sb, 1, 1, 1))

            # ---- final projection: one (qm, jh) half ----
            ost_tiles = {}
            opool_toggle = {}

            def emit_final_half(qb, qm, jh, pool=None, tag=None,
                                evict_act=False):
                pool = pool or psO
                tag = tag or "O"
                ops = pool.tile([128, 512], f32, tag=tag,
                                name=f"ops{qb}{qm}{jh}")
                for t in range(2):
                    nc.tensor.matmul(
                        ops,
                        ot_alls[qb][:, t, ts(qm, 128)],
                        wo_sb[:, t, ts(jh, 512)],
                        start=(t == 0),
                        stop=(t == 1),
                    )
                if (qb, qm) not in ost_tiles:
                    ost_tiles[(qb, qm)] = stream.tile(
                        [128, Q_DIM], bf16, tag="ost", bufs=3,
                        name=f"ost{qb}{qm}"
                    )
                ost = ost_tiles[(qb, qm)]
                if evict_act:
                    nc.scalar.copy(out=ost[:, ts(jh, 512)], in_=ops)
                else:
                    nc.vector.tensor_copy(out=ost[:, ts(jh, 512)], in_=ops)
                if jh == 1:
                    nc.sync.dma_start(
                        out=out_d[ds(qb * QB + qm * 128, 128), :], in_=ost
                    )

            # ---- attention state ----
            ot_alls = {}
            pv_banks = {}    # (qb, h) -> [tile_a, tile_b]
            pk_tiles = {}    # (qb, pair) -> packed normalized tile
            pts = {}         # step -> pt tile
            sts = {}         # step -> score tile (act) or sbuf copy (pool)

            steps = [(qb, h, kc)
                     for qb in range(N_QB)
                     for h in range(H_PER_CORE)
                     for kc in range(KC)]
            NSTEP = len(steps)

            def is_pool(qb, h, kc):
                return kc in POOL_KCS and (qb, h) not in NO_POOL_HEADS

            def lead_of(qb, h, kc):
                if is_pool(qb, h, kc):
                    return PV_LEAD_POOL
                # stagger the first PVs of each head so they never wait on
                # the previous head's normalize reads (psPV is single-buffered)
                return {0: 3, 1: 2}.get(kc, PV_LEAD_ACT)

            # per-step PV due lists: pv_due[g] = list of source steps
            pv_due = [[] for _ in range(NSTEP + PV_LEAD_POOL + 1)]
            for g, (qb, h, kc) in enumerate(steps):
                pv_due[g + lead_of(qb, h, kc)].append(g)
            # per-(qb,h) bank emission bookkeeping: which (kc, j) is first/
            # last per bank in emission order.
            bank_seq = {}  # (qb, h, bank) -> [positions...] as (kc, j)
            for g, (qb, h, kc) in enumerate(steps):
                for j in range(8):
                    bank_seq.setdefault((qb, h, j // 4), []).append(
                        (g + lead_of(qb, h, kc), kc, j))
            for key in bank_seq:
                bank_seq[key].sort()
            bank_first = {k: v[0] for k, v in bank_seq.items()}
            bank_last = {k: v[-1] for k, v in bank_seq.items()}

            def emit_S(g):
                qb, h, kc = steps[g]
                t, po = h // 2, (h % 2) * D
                if is_pool(qb, h, kc):
                    # pool-path scores live outside the psS ring: one 512
                    # half in psT, one in psO
                    st = (
                        psT.tile([128, 512], f32, tag="T", name=f"sp{qb}{h}{kc}a"),
                        psO.tile([128, 512], f32, tag="O", name=f"sp{qb}{h}{kc}b"),
                    )
                    for j in range(2):
                        nc.tensor.matmul(
                            st[j],
                            kT_sb[po : po + D, t, ts(kc, 128)],
                            qT_tiles[qb][po : po + D, t, ts(j, 512)],
                            start=True,
                            stop=True,
                        )
                    return st
                st = psS.tile([128, QB], f32, tag="S", name=f"st{qb}{h}{kc}")
                for j in range(2):
                    nc.tensor.matmul(
                        st[:, ts(j, 512)],
                        kT_sb[po : po + D, t, ts(kc, 128)],
                        qT_tiles[qb][po : po + D, t, ts(j, 512)],
                        start=True,
                        stop=True,
                    )
                return st

            dbg_pt_tile = {}

            def emit_exp(g):
                qb, h, kc = steps[g]
                pt = stream.tile([128, QB], bf16, tag="pt", bufs=6,
                                 name=f"pt{qb}{h}{kc}")
                if _DEBUG and g == 0:
                    dbg_pt_tile["t"] = stream.tile([128, QB], bf16, tag="dbgpt",
                                                   bufs=1, name="dbgpt")
                st = sts.pop(g)
                if is_pool(qb, h, kc):
                    st_sb = stream.tile([128, QB], f32, tag="stsb", bufs=2,
                                        name=f"stsb{qb}{h}{kc}")
                    nc.vector.tensor_copy(out=st_sb[:, 0:512], in_=st[0])
                    nc.vector.tensor_copy(out=st_sb[:, 512:1024], in_=st[1])
                    nc.gpsimd.tensor_tensor(out=pt, in0=base, in1=st_sb,
                                            op=Pow)
                elif g == 0:
                    # split halves: exp(j0) can run before x0-j1 has landed
                    for j in range(2):
                        nc.scalar.activation(out=pt[:, ts(j, 512)],
                                             in_=st[:, ts(j, 512)],
                                             func=Exp, scale=SCALE)
                else:
                    nc.scalar.activation(out=pt, in_=st, func=Exp,
                                         scale=SCALE)
                if _DEBUG and g == 0:
                    nc.vector.tensor_copy(out=dbg_pt_tile["t"], in_=pt)
                pts[g] = pt

            def emit_PV(src_g):
                qb, h, kc = steps[src_g]
                if (qb, h) not in pv_banks:
                    pv_banks[(qb, h)] = [
                        psPV.tile([128, 4, D + 1], f32, tag=f"pv{bk}",
                                  name=f"pv{qb}{h}{bk}")
                        for bk in range(2)
                    ]
                pt = pts.pop(src_g)
                banks = pv_banks[(qb, h)]
                for j in range(8):
                    bk = j // 4
                    nc.tensor.matmul(
                        banks[bk][:, j % 4, :],
                        pt[:, ts(j, 128)],
                        v_sb[:, kc, h, :],
                        start=((kc, j) == bank_first[(qb, h, bk)][1:]),
                        stop=((kc, j) == bank_last[(qb, h, bk)][1:]),
                    )

            def emit_norm(qb, h):
                """recip + normalize-evict into the packed pair tile."""
                banks = pv_banks.pop((qb, h))
                pair, slot = h // 2, h % 2
                if (qb, pair) not in pk_tiles:
                    pk_tiles[(qb, pair)] = stream.tile(
                        [128, 8, 2, D], bf16, tag="pk", bufs=2,
                        name=f"pk{qb}{pair}"
                    )
                pk = pk_tiles[(qb, pair)]
                recip = stream.tile([128, 8], f32, tag="recip", bufs=2,
                                    name=f"rc{qb}{h}")
                for bk in range(2):
                    nc.vector.reciprocal(out=recip[:, ds(bk * 4, 4)],
                                         in_=banks[bk][:, :, D])
                for j in range(8):
                    nc.vector.tensor_scalar_mul(
                        pk[:, j, slot, :],
                        banks[j // 4][:, j % 4, 0:D],
                        recip[:, ds(j, 1)],
                    )

            def emit_transpose(qb, pair):
                pk = pk_tiles.pop((qb, pair))
                trp = psT.tile([128, 8, 128], bf16, tag="T",
                               name=f"trp{qb}{pair}")
                for qc in range(8):
                    nc.tensor.matmul(
                        trp[:, qc, :],
                        pk[:, qc, :, :],
                        ident_sb,
                        start=True,
                        stop=True,
                        is_transpose=True,
                    )
                if qb not in ot_alls:
                    ot_alls[qb] = stream.tile([128, 2, QB], bf16,
                                              tag="otall", bufs=2,
                                              name=f"otall{qb}")
                nc.vector.tensor_copy(
                    out=ot_alls[qb][:, pair, :],
                    in_=trp.rearrange("p a b -> p (a b)"),
                )

            # outproj(qb0) fillers bind to qb1's pool steps: emitted two
            # steps before pool-kc p, the psO tag chain delays their matmuls
            # until poolS(p) is copied, so the Act eviction lands exactly in
            # pool step p's Act idle.
            for qm in range(8):
                for jh in range(2):
                    slot = qm * 2 + jh  # 0..15
                    add_fill(1, slot // 4, POOL_KCS[slot % 4] - 2,
                             lambda qm=qm, jh=jh: emit_final_half(
                                 0, qm, jh, evict_act=True))

            # ---- flat software-pipelined stream ----
            sts[0] = emit_S(0)
            sts[1] = emit_S(1)
            for g, (qb, h, kc) in enumerate(steps):
                if g + 2 < NSTEP:
                    sts[g + 2] = emit_S(g + 2)
                emit_exp(g)
                for src in pv_due[g]:
                    emit_PV(src)
                # head completion: when the last PV emission position for
                # (qb', h') was at this g, normalize (and transpose on pairs)
                for key, (lg, lkc, lj) in list(bank_last.items()):
                    pqb, ph, pbk = key
                    if pbk == 1 and lg == g:
                        emit_norm(pqb, ph)
                        if ph % 2 == 1:
                            emit_transpose(pqb, ph // 2)
                for fn in fill.pop((qb, h, kc), ()):
                    fn()

            # ---- drain ----
            for g in range(NSTEP, NSTEP + PV_LEAD_POOL + 1):
                for src in pv_due[g]:
                    emit_PV(src)
                for key, (lg, lkc, lj) in list(bank_last.items()):
                    pqb, ph, pbk = key
                    if pbk == 1 and lg == g:
                        emit_norm(pqb, ph)
                        if ph % 2 == 1:
                            emit_transpose(pqb, ph // 2)

            if _DEBUG:
                nc.sync.dma_start(out=dbg_kt[:, :, :], in_=kT_sb)
                nc.sync.dma_start(out=dbg_qt[:, :, :], in_=qT0_sb)
                nc.sync.dma_start(out=dbg_v[:, :, :, :], in_=v_sb)
                nc.sync.dma_start(out=dbg_ot[:, :, :], in_=ot_alls[0])
                nc.sync.dma_start(out=dbg_pt[:, :], in_=dbg_pt_tile["t"])

            # tail: qb1 output projection round-robins over every psum
            # bank that is free after the last normalize (psS x2, psPV x2,
            # psT, psO), evictions alternating DVE/Act, so the groups and
            # their DMAs pipeline ~6 deep.
            tail_pools = [(psS, "S"), (psPV, "pv0"), (psT, "T"),
                          (psS, "S"), (psPV, "pv1"), (psO, "O")]
            for i, (qm, jh) in enumerate(
                    (qm, jh) for qm in range(8) for jh in range(2)):
                pool, tag = tail_pools[i % len(tail_pools)]
                emit_final_half(1, qm, jh, pool=pool, tag=tag,
                                evict_act=(i % 2 == 1))

    nc.finalize()
    return nc


def _get_nc():
    global _CACHED_NC
    if _CACHED_NC is None:
        _CACHED_NC = _build_bass()
    return _CACHED_NC


def _numpy_fallback(x, context, mask, Wq, Wk, Wv, Wout, bout):
    q = (x @ Wq.T).reshape(B, N, H, D)
    k = (context @ Wk.T).reshape(B, M, H, D)
    v = (context @ Wv.T).reshape(B, M, H, D)
    sim = np.einsum("bnhd,bmhd->bhnm", q, k) * SCALE
    sim = np.where(mask[:, None, None, :], sim, -np.finfo(np.float32).max)
    sim -= sim.max(axis=-1, keepdims=True)
    attn = np.exp(sim)
    attn /= attn.sum(axis=-1, keepdims=True)
    out = np.einsum("bhnm,bmhd->bnhd", attn, v).reshape(B, N, INNER)
    return (out @ Wout.T + bout).astype(np.float32)


def kernel(x, context, mask, Wq, Wk, Wv, Wout, bout, _want_results=False):
    import ml_dtypes

    bf = ml_dtypes.bfloat16
    x = np.asarray(x, dtype=np.float32)
    context = np.asarray(context, dtype=np.float32)
    mask = np.asarray(mask)
    Wq = np.asarray(Wq, dtype=np.float32)
    Wk = np.asarray(Wk, dtype=np.float32)
    Wv = np.asarray(Wv, dtype=np.float32)
    Wout = np.asarray(Wout, dtype=np.float32)
    bout = np.asarray(bout, dtype=np.float32)

    if not mask.all():
        return _numpy_fallback(x, context, mask, Wq, Wk, Wv, Wout, bout)

    from concourse.bass_utils import run_bass_kernel_spmd

    ident = np.eye(128, dtype=np.float32).astype(bf)
    in_maps = []
    for c in range(N_CORES):
        b, hg = c // 2, c % 2
        sl = slice(hg * IN_PER_CORE, (hg + 1) * IN_PER_CORE)
        in_maps.append(
            {
                "xT": np.ascontiguousarray(x[b].T).astype(bf),
                "cT": np.ascontiguousarray(context[b].T).astype(bf),
                "wq": np.ascontiguousarray(
                    Wq[sl, :].T.reshape(8, 128, 256).transpose(1, 0, 2)
                ).astype(bf),
                "wk": np.ascontiguousarray(
                    Wk[sl, :].T.reshape(6, 128, 256).transpose(1, 0, 2)
                ).astype(bf),
                "wv": np.ascontiguousarray(
                    Wv[sl, :].T.reshape(6, 128, 256).transpose(1, 0, 2)
                ).astype(bf),
                "wo": np.ascontiguousarray(
                    Wout[:, sl].T.reshape(2, 128, 1024).transpose(1, 0, 2)
                ).astype(bf),
                "ident": ident,
            }
        )

    res = run_bass_kernel_spmd(_get_nc(), in_maps, core_ids=list(range(N_CORES)))

    out = np.empty((B, N, Q_DIM), dtype=np.float32)
    for b in range(B):
        out[b] = (res.results[2 * b]["out"].astype(np.float32)
                  + res.results[2 * b + 1]["out"].astype(np.float32) + bout)
    if _want_results:
        return out, res
    return out


# revision 10
# speedup vs baseline: 1.1854x; 1.0393x over previous
"""Trainium2 Bass kernel for CrossAttention (B=4, N=M=2048, H=8, D=64,
Q_DIM=1024, C_DIM=768).  v2: bf16 datapath + q-partitioned PV + split exp.

Sharding over 8 cores: core c handles batch b = c//2 and head-group
hg = c%2 (4 heads, 256 inner dims).  Each core computes a *partial*
output projection; the host sums core pairs and adds the output bias.

Key structure (chosen against the concourse TimelineSim cost model):
  - all matmul operands bf16 (1 cycle/row at any width); accumulation f32.
  - scores S.T[keys, q] per (qb, h, kc): 2 ap-512 matmuls (K=64).
  - PV is q-partitioned: out[128q, 65] per q-chunk with pt chunk stationary
    and v (with an appended ones-column -> softmax denominators) moving:
    520 cols per kc step instead of 1024 -> half the PE cost of the
    keys-partitioned form.  The 8 q-chunk accumulation groups share two
    PSUM banks via a single start/stop per bank (start marks the whole
    2KB zero region; first touch of each chunk overwrites).
  - exp is split: ~2/3 of score tiles on Act (activation Exp), ~1/3 via
    DVE copy to SBUF + GPSIMD pow(e^SCALE, S) (GPSIMD cannot read PSUM).
    Pool-path PV consumption is deferred 3 steps so the PE never waits.
  - normalization folds into the PV eviction: DVE reciprocal of the
    denominator column, then tensor_scalar_mul into packed bf16 tiles.
  - packed [128q, 128inner] head-pair tiles are PE-transposed (identity
    permutation rhs) so the output projection gets inner-contracted lhsT.
  - output projection per 128-query chunk: 2x2 ap-512 matmuls, evicted to
    SBUF and DMA'd per chunk.

The attention mask in this problem is all-True; if a mask with False
entries is ever passed, kernel() falls back to a numpy reference.
"""

import numpy as np

B, N, M = 4, 2048, 2048
Q_DIM, C_DIM, H, D = 1024, 768, 8, 64
INNER = H * D  # 512
SCALE = D ** -0.5

N_CORES = 8
W_SCALE = 32.0  # fp8 weight pre-scale so residuals clear the e4m3 subnormal floor
H_PER_CORE = 4
IN_PER_CORE = H_PER_CORE * D  # 256
QB = 1024
N_QB = N // QB          # 2
KC = M // 128           # 16 key chunks
QK_CHUNKS = Q_DIM // 128   # 8
CK_CHUNKS = C_DIM // 128   # 6

# key-chunks whose exp runs on GPSIMD (via DVE psum->sbuf copy); the rest
# run on Act.  Pool-path score tiles live OUTSIDE the 2-deep psS ring (one
# 512 half each in psT/psO) so the Act exp chain never waits on the copy;
# their PV consumption is deferred 4 steps.  The two DMA-paced early heads
# stay all-Act.
POOL_KCS = (3, 6, 9, 11)
NO_POOL_HEADS = ((0, 0), (0, 1))
PV_LEAD_ACT = 1
PV_LEAD_POOL = 4

_CACHED_NC = None
_DEBUG = False


def _build_bass():
    import concourse.bass as bass
    import concourse.mybir as mybir
    import concourse.tile as tile
    from concourse import bacc

    f32 = mybir.dt.float32
    f32r = mybir.dt.float32r
    bf16 = mybir.dt.bfloat16
    ts, ds = bass.ts, bass.ds
    Exp = mybir.ActivationFunctionType.Exp
    Pow = mybir.AluOpType.pow

    nc = bacc.Bacc("TRN2", target_bir_lowering=False)

    # all projection inputs arrive as fp8 hi/lo pairs, pre-arranged in
    # their DoubleRow SBUF layouts (contraction-chunk pairs on a free dim)
    fp8 = mybir.dt.float8e4
    x8_d = nc.dram_tensor("x8", [128, 4, 2, N], fp8, kind="ExternalInput")
    xr_d = nc.dram_tensor("xr", [128, 4, 2, N], fp8, kind="ExternalInput")
    c8_d = nc.dram_tensor("c8", [128, 3, 2, M], fp8, kind="ExternalInput")
    cr_d = nc.dram_tensor("cr", [128, 3, 2, M], fp8, kind="ExternalInput")
    wq8_d = nc.dram_tensor("wq8", [128, 4, 2, IN_PER_CORE], fp8, kind="ExternalInput")
    wqr_d = nc.dram_tensor("wqr", [128, 4, 2, IN_PER_CORE], fp8, kind="ExternalInput")
    wk8_d = nc.dram_tensor("wk8", [128, 3, 2, IN_PER_CORE], fp8, kind="ExternalInput")
    wkr_d = nc.dram_tensor("wkr", [128, 3, 2, IN_PER_CORE], fp8, kind="ExternalInput")
    wv8_d = nc.dram_tensor("wv8", [128, 3, 2, IN_PER_CORE], fp8, kind="ExternalInput")
    wvr_d = nc.dram_tensor("wvr", [128, 3, 2, IN_PER_CORE], fp8, kind="ExternalInput")
    wo = nc.dram_tensor("wo", [128, 2, Q_DIM], bf16, kind="ExternalInput")
    ident_d = nc.dram_tensor("ident", [128, 128], bf16, kind="ExternalInput")
    out_d = nc.dram_tensor("out", [N, Q_DIM], bf16, kind="ExternalOutput")
    if _DEBUG:
        dbg_kt = nc.dram_tensor("dbg_kt", [128, 2, M], bf16, kind="ExternalOutput")
        dbg_qt = nc.dram_tensor("dbg_qt", [128, 2, QB], bf16, kind="ExternalOutput")
        dbg_v = nc.dram_tensor("dbg_v", [128, KC, H_PER_CORE, D + 1], bf16, kind="ExternalOutput")
        dbg_ot = nc.dram_tensor("dbg_ot", [128, 2, QB], bf16, kind="ExternalOutput")
        dbg_pt = nc.dram_tensor("dbg_pt", [128, QB], bf16, kind="ExternalOutput")

    with tile.TileContext(nc) as tc:
        with (
            tc.tile_pool(name="persist", bufs=1) as persist,
            tc.tile_pool(name="stream", bufs=2) as stream,
            tc.tile_pool(name="psS", bufs=2, space="PSUM") as psS,
            tc.tile_pool(name="psPV", bufs=1, space="PSUM") as psPV,
            tc.tile_pool(name="psT", bufs=1, space="PSUM") as psT,
            tc.tile_pool(name="psO", bufs=1, space="PSUM") as psO,
        ):
            # ---- constants ----
            onesb = persist.tile([128, 64], bf16, tag="onesb")
            nc.vector.memset(onesb, 1.0)
            base = persist.tile([128, QB], f32, tag="base")
            nc.vector.memset(base, float(np.exp(SCALE)))
            wrm = persist.tile([128, 512], bf16, tag="wrm")
            nc.vector.memset(wrm, 0.0)
            # warm the Act exp table during the DMA shadow
            warm2 = persist.tile([128, 1], f32, tag="warm2")
            nc.scalar.activation(out=warm2, in_=wrm[:, 0:1], func=Exp,
                                 scale=SCALE)

            # PE p-state warmup: keep the tensor engine continuously busy
            # through the initial DMA wait so the first real matmuls run at
            # full clock (the cost model ramps 0.65->1.2->2.4 GHz over 3us
            # of continuous execution).
            warm_ps = psO.tile([128, 512], f32, tag="O", name="warmps")
            N_WARM = 13
            for i in range(N_WARM):
                nc.tensor.matmul(warm_ps, wrm[:, 0:128], wrm,
                                 start=(i == 0), stop=(i == N_WARM - 1))

            # ---- DMA schedule (execution order == emission order) ----
            # qT chain first (wq + x0-j0), then kT chain (wk + ctx-q0).
            ctx_sb = persist.tile([128, 2, 3, 2, M], fp8, tag="ctx")

            def dma_ctx_quarter(q):
                nc.sync.dma_start(
                    out=ctx_sb[:, 0, :, :, ds(q * 512, 512)],
                    in_=c8_d[:, :, :, ds(q * 512, 512)],
                )
                nc.sync.dma_start(
                    out=ctx_sb[:, 1, :, :, ds(q * 512, 512)],
                    in_=cr_d[:, :, :, ds(q * 512, 512)],
                )

            wq_sb = persist.tile([128, 2, 4, 2, IN_PER_CORE], fp8, tag="wq")
            nc.sync.dma_start(out=wq_sb[:, 0], in_=wq8_d[:, :, :, :])

            # x0 split by j-half so qT(qb0, m0, j0) completes early.
            # hl dim: 0 = fp8 high part, 1 = fp8 residual.
            x0_sb = stream.tile([128, 2, 4, 2, QB], fp8, tag="x0", bufs=1,
                                name="x0")
            nc.sync.dma_start(out=x0_sb[:, 0, :, :, 0:512],
                              in_=x8_d[:, :, :, 0:512])
            nc.sync.dma_start(out=wq_sb[:, 1], in_=wqr_d[:, :, :, :])
            nc.sync.dma_start(out=x0_sb[:, 1, :, :, 0:512],
                              in_=xr_d[:, :, :, 0:512])

            wk_sb = persist.tile([128, 2, 3, 2, IN_PER_CORE], fp8, tag="wk")
            nc.sync.dma_start(out=wk_sb[:, 0], in_=wk8_d[:, :, :, :])
            nc.sync.dma_start(out=wk_sb[:, 1], in_=wkr_d[:, :, :, :])

            dma_ctx_quarter(0)

            nc.sync.dma_start(out=x0_sb[:, 0, :, :, 512:1024],
                              in_=x8_d[:, :, :, 512:1024])
            nc.sync.dma_start(out=x0_sb[:, 1, :, :, 512:1024],
                              in_=xr_d[:, :, :, 512:1024])

            wv_sb = persist.tile([128, 2, 3, 2, IN_PER_CORE], fp8, tag="wv")
            nc.sync.dma_start(out=wv_sb[:, 0], in_=wv8_d[:, :, :, :])
            nc.sync.dma_start(out=wv_sb[:, 1], in_=wvr_d[:, :, :, :])

            dma_ctx_quarter(1)
            dma_ctx_quarter(2)
            dma_ctx_quarter(3)

            x1_sb = stream.tile([128, 2, 4, 2, QB], fp8, tag="x1", bufs=1,
                                name="x1")
            nc.sync.dma_start(out=x1_sb[:, 0], in_=x8_d[:, :, :, QB : 2 * QB])
            nc.sync.dma_start(out=x1_sb[:, 1], in_=xr_d[:, :, :, QB : 2 * QB])

            ident_sb = persist.tile([128, 128], bf16, tag="ident")
            nc.sync.dma_start(out=ident_sb, in_=ident_d[:, :])

            wo_sb = persist.tile([128, 2, Q_DIM], bf16, tag="wo")
            nc.sync.dma_start(out=wo_sb, in_=wo[:, :, :])

            # ---- persistent compute targets ----
            kT_sb = persist.tile([128, 2, M], bf16, tag="kt")
            v_sb = persist.tile([128, KC, H_PER_CORE, D + 1], bf16, tag="v")
            nc.vector.tensor_copy(
                out=v_sb[:, :, :, D : D + 1],
                in_=onesb.rearrange("p (a b c) -> p a b c", a=KC, b=H_PER_CORE),
            )

            # ---- projection pieces (3-term fp8 DoubleRow: hi*hi, hi*lo,
            # lo*hi; the lo*lo term is ~1e-6 relative and dropped) ----
            DR = mybir.MatmulPerfMode.DoubleRow
            TERMS = ((0, 0), (0, 1), (1, 0))

            def emit_kt_q(m, q, pool, tag):
                """kT_sb[:, m, q*512:(q+1)*512]: 9 DR matmuls."""
                kq = pool.tile([128, 512], f32, tag=tag, name=f"kq{m}{q}")
                n = 0
                for tw, tx in TERMS:
                    for pr in range(3):
                        n += 1
                        nc.tensor.matmul(
                            kq,
                            wk_sb[:, tw, pr, :, ts(m, 128)],
                            ctx_sb[:, tx, pr, :, ds(q * 512, 512)],
                            start=(n == 1),
                            stop=(n == 9),
                            perf_mode=DR,
                        )
                nc.vector.tensor_scalar_mul(kT_sb[:, m, ts(q, 512)], kq,
                                             1.0 / W_SCALE)

            _vps_open = {}

            def emit_v_chunk(kc, part=None):
                """part=None: whole chunk; part=0/1: half the contraction
                (the psum tile stays open between the two halves, so no
                other psT user may be emitted in between)."""
                if part in (None, 0):
                    # padded to 2KB so every psT tile shares one size class
                    _vps_open[kc] = psT.tile([128, 512], f32, tag="T",
                                             name=f"vps{kc}")
                vps = _vps_open[kc]
                lo, hi = {None: (0, 9), 0: (0, 5), 1: (5, 9)}[part]
                pieces = [(tw, tx, pr) for tw, tx in TERMS for pr in range(3)]
                for n in range(lo, hi):
                    tw, tx, pr = pieces[n]
                    nc.tensor.matmul(
                        vps[:, 0:IN_PER_CORE],
                        ctx_sb[:, tx, pr, :, ts(kc, 128)],
                        wv_sb[:, tw, pr, :, :],
                        start=(n == 0),
                        stop=(n == 8),
                        perf_mode=DR,
                    )
                if part in (None, 1):
                    nc.vector.tensor_scalar_mul(
                        v_sb[:, kc, :, 0:D],
                        _vps_open.pop(kc)[:, 0:IN_PER_CORE].rearrange(
                            "p (h d) -> p h d", h=H_PER_CORE),
                        1.0 / W_SCALE,
                    )

            # atomic qT j-half (8 matmuls + evict; single-buffer pools need
            # each user to fully retire before the next opens)
            def emit_qt_j(qT_sb, x_sb, m, qb, jh, pool=None, tag=None):
                pool = pool or psO
                tag = tag or "O"
                qps = pool.tile([128, 512], f32, tag=tag,
                                name=f"qj{qb}{m}{jh}")
                n = 0
                for tw, tx in TERMS:
                    for pr in range(4):
                        n += 1
                        nc.tensor.matmul(
                            qps,
                            wq_sb[:, tw, pr, :, ts(m, 128)],
                            x_sb[:, tx, pr, :, ds(jh * 512, 512)],
                            start=(n == 1),
                            stop=(n == 12),
                            perf_mode=DR,
                        )
                nc.vector.tensor_scalar_mul(qT_sb[:, m, ts(jh, 512)], qps,
                                             1.0 / W_SCALE)

            # ---- prologue (PE order matches DMA arrival order) ----
            qT0_sb = stream.tile([128, 2, QB], bf16, tag="qt0", bufs=1,
                                 name="qT0")
            emit_qt_j(qT0_sb, x0_sb, 0, 0, 0, pool=psS, tag="S")
            emit_kt_q(0, 0, psS, "S")
            emit_v_chunk(0)
            emit_v_chunk(1)
            emit_qt_j(qT0_sb, x0_sb, 0, 0, 1, pool=psS, tag="S")
            emit_v_chunk(2)
            emit_v_chunk(3)

            qT1_sb = stream.tile([128, 2, QB], bf16, tag="qt1", bufs=1,
                                 name="qT1")
            qT_tiles = [qT0_sb, qT1_sb]

            # ---- filler slot table: fill[(qb, h, kc)] -> list of fns ----
            fill = {}

            def add_fill(qb, h, kc, fn):
                fill.setdefault((qb, h, kc), []).append(fn)

            # Fillers are atomic (open + matmuls + evict) and spaced >=3
            # steps apart so the 2-deep score ring / Act exp queue absorbs
            # each clump before the next.  v chunks in h0 are split halves
            # (psT has no other user there).
            add_fill(0, 0, 1, lambda: emit_kt_q(0, 1, psO, "O"))
            add_fill(0, 0, 5, lambda: emit_kt_q(0, 2, psO, "O"))
            add_fill(0, 0, 9, lambda: emit_kt_q(0, 3, psO, "O"))
            for kc0, slot in ((4, 1), (5, 2), (6, 3), (7, 4), (8, 5), (9, 6),
                              (10, 7), (11, 8), (12, 9), (13, 10), (14, 11),
                              (15, 13)):
                add_fill(0, 0, slot, lambda kc=kc0: emit_v_chunk(kc, 0))
                add_fill(0, 0, slot + 1, lambda kc=kc0: emit_v_chunk(kc, 1))
            # kT(m1, q0..q3) gate S(h2, *): fill h1 slots (ctx all landed).
            add_fill(0, 1, 1, lambda: emit_kt_q(1, 0, psO, "O"))
            add_fill(0, 1, 4, lambda: emit_kt_q(1, 1, psO, "O"))
            add_fill(0, 1, 7, lambda: emit_kt_q(1, 2, psO, "O"))
            add_fill(0, 1, 10, lambda: emit_kt_q(1, 3, psO, "O"))
            # qT(qb0, m1): gates S(h2) (emitted step 30).
            add_fill(0, 1, 12, lambda: emit_qt_j(qT0_sb, x0_sb, 1, 0, 0))
            add_fill(0, 1, 13, lambda: emit_qt_j(qT0_sb, x0_sb, 1, 0, 1))
            # qT(qb1, m0): gates S(qb1, h0) (emitted step 62); x1 by ~28us.
            add_fill(0, 2, 2, lambda: emit_qt_j(qT1_sb, x1_sb, 0, 1, 0))
            add_fill(0, 2, 8, lambda: emit_qt_j(qT1_sb, x1_sb, 0, 1, 1))
            # qT(qb1, m1): gates S(qb1, h2) (emitted step 94): qb1-h0 slots.
            add_fill(1, 0, 11, lambda: emit_qt_j(qT1_sb, x1_sb, 1, 1, 0))
            add_fill(1, 0, 13, lambda: emit_qt_j(qT1_sb, x1_sb, 1, 1, 1))

            # ---- final projection: one (qm, jh) half ----
            ost_tiles = {}
            opool_toggle = {}

            def emit_final_half(qb, qm, jh, pool=None, tag=None,
                                evict_act=False):
                pool = pool or psO
                tag = tag or "O"
                ops = pool.tile([128, 512], f32, tag=tag,
                                name=f"ops{qb}{qm}{jh}")
                for t in range(2):
                    nc.tensor.matmul(
                        ops,
                        ot_alls[qb][:, t, ts(qm, 128)],
                        wo_sb[:, t, ts(jh, 512)],
                        start=(t == 0),
                        stop=(t == 1),
                    )
                if (qb, qm) not in ost_tiles:
                    ost_tiles[(qb, qm)] = stream.tile(
                        [128, Q_DIM], bf16, tag="ost", bufs=3,
                        name=f"ost{qb}{qm}"
                    )
                ost = ost_tiles[(qb, qm)]
                if evict_act:
                    nc.scalar.copy(out=ost[:, ts(jh, 512)], in_=ops)
                else:
                    nc.vector.tensor_copy(out=ost[:, ts(jh, 512)], in_=ops)
                if jh == 1:
                    nc.sync.dma_start(
                        out=out_d[ds(qb * QB + qm * 128, 128), :], in_=ost
                    )

            # ---- attention state ----
            ot_alls = {}
            pv_banks = {}    # (qb, h) -> [tile_a, tile_b]
            pk_tiles = {}    # (qb, pair) -> packed normalized tile
            pts = {}         # step -> pt tile
            sts = {}         # step -> score tile (act) or sbuf copy (pool)

            steps = [(qb, h, kc)
                     for qb in range(N_QB)
                     for h in range(H_PER_CORE)
                     for kc in range(KC)]
            NSTEP = len(steps)

            def is_pool(qb, h, kc):
                return kc in POOL_KCS and (qb, h) not in NO_POOL_HEADS

            def lead_of(qb, h, kc):
                if is_pool(qb, h, kc):
                    return PV_LEAD_POOL
                # stagger the first PVs of each head so they never wait on
                # the previous head's normalize reads (psPV is single-buffered)
                return {0: 3, 1: 2}.get(kc, PV_LEAD_ACT)

            # per-step PV due lists: pv_due[g] = list of source steps
            pv_due = [[] for _ in range(NSTEP + PV_LEAD_POOL + 1)]
            for g, (qb, h, kc) in enumerate(steps):
                pv_due[g + lead_of(qb, h, kc)].append(g)
            # per-(qb,h) bank emission bookkeeping: which (kc, j) is first/
            # last per bank in emission order.
            bank_seq = {}  # (qb, h, bank) -> [positions...] as (kc, j)
            for g, (qb, h, kc) in enumerate(steps):
                for j in range(8):
                    bank_seq.setdefault((qb, h, j // 4), []).append(
                        (g + lead_of(qb, h, kc), kc, j))
            for key in bank_seq:
                bank_seq[key].sort()
            bank_first = {k: v[0] for k, v in bank_seq.items()}
            bank_last = {k: v[-1] for k, v in bank_seq.items()}

            def emit_S(g):
                qb, h, kc = steps[g]
                t, po = h // 2, (h % 2) * D
                if is_pool(qb, h, kc):
                    # pool-path scores live outside the psS ring: one 512
                    # half in psT, one in psO
                    st = (
                        psT.tile([128, 512], f32, tag="T", name=f"sp{qb}{h}{kc}a"),
                        psO.tile([128, 512], f32, tag="O", name=f"sp{qb}{h}{kc}b"),
                    )
                    for j in range(2):
                        nc.tensor.matmul(
                            st[j],
                            kT_sb[po : po + D, t, ts(kc, 128)],
                            qT_tiles[qb][po : po + D, t, ts(j, 512)],
                            start=True,
                            stop=True,
                        )
                    return st
                st = psS.tile([128, QB], f32, tag="S", name=f"st{qb}{h}{kc}")
                for j in range(2):
                    nc.tensor.matmul(
                        st[:, ts(j, 512)],
                        kT_sb[po : po + D, t, ts(kc, 128)],
                        qT_tiles[qb][po : po + D, t, ts(j, 512)],
                        start=True,
                        stop=True,
                    )
                return st

            dbg_pt_tile = {}

            def emit_exp(g):
                qb, h, kc = steps[g]
                pt = stream.tile([128, QB], bf16, tag="pt", bufs=6,
                                 name=f"pt{qb}{h}{kc}")
                if _DEBUG and g == 0:
                    dbg_pt_tile["t"] = stream.tile([128, QB], bf16, tag="dbgpt",
                                                   bufs=1, name="dbgpt")
                st = sts.pop(g)
                if is_pool(qb, h, kc):
                    st_sb = stream.tile([128, QB], f32, tag="stsb", bufs=2,
                                        name=f"stsb{qb}{h}{kc}")
                    nc.vector.tensor_copy(out=st_sb[:, 0:512], in_=st[0])
                    nc.vector.tensor_copy(out=st_sb[:, 512:1024], in_=st[1])
                    nc.gpsimd.tensor_tensor(out=pt, in0=base, in1=st_sb,
                                            op=Pow)
                elif g == 0:
                    # split halves: exp(j0) can run before x0-j1 has landed
                    for j in range(2):
                        nc.scalar.activation(out=pt[:, ts(j, 512)],
                                             in_=st[:, ts(j, 512)],
                                             func=Exp, scale=SCALE)
                else:
                    nc.scalar.activation(out=pt, in_=st, func=Exp,
                                         scale=SCALE)
                if _DEBUG and g == 0:
                    nc.vector.tensor_copy(out=dbg_pt_tile["t"], in_=pt)
                pts[g] = pt

            def emit_PV(src_g):
                qb, h, kc = steps[src_g]
                if (qb, h) not in pv_banks:
                    pv_banks[(qb, h)] = [
                        psPV.tile([128, 4, D + 1], f32, tag=f"pv{bk}",
                                  name=f"pv{qb}{h}{bk}")
                        for bk in range(2)
                    ]
                pt = pts.pop(src_g)
                banks = pv_banks[(qb, h)]
                for j in range(8):
                    bk = j // 4
                    nc.tensor.matmul(
                        banks[bk][:, j % 4, :],
                        pt[:, ts(j, 128)],
                        v_sb[:, kc, h, :],
                        start=((kc, j) == bank_first[(qb, h, bk)][1:]),
                        stop=((kc, j) == bank_last[(qb, h, bk)][1:]),
                    )

            def emit_norm(qb, h):
                """recip + normalize-evict into the packed pair tile."""
                banks = pv_banks.pop((qb, h))
                pair, slot = h // 2, h % 2
                if (qb, pair) not in pk_tiles:
                    pk_tiles[(qb, pair)] = stream.tile(
                        [128, 8, 2, D], bf16, tag="pk", bufs=2,
                        name=f"pk{qb}{pair}"
                    )
                pk = pk_tiles[(qb, pair)]
                recip = stream.tile([128, 8], f32, tag="recip", bufs=2,
                                    name=f"rc{qb}{h}")
                for bk in range(2):
                    nc.vector.reciprocal(out=recip[:, ds(bk * 4, 4)],
                                         in_=banks[bk][:, :, D])
                for j in range(8):
                    nc.vector.tensor_scalar_mul(
                        pk[:, j, slot, :],
                        banks[j // 4][:, j % 4, 0:D],
                        recip[:, ds(j, 1)],
                    )

            def emit_transpose(qb, pair):
                pk = pk_tiles.pop((qb, pair))
                trp = psT.tile([128, 8, 128], bf16, tag="T",
                               name=f"trp{qb}{pair}")
                for qc in range(8):
                    nc.tensor.matmul(
                        trp[:, qc, :],
                        pk[:, qc, :, :],
                        ident_sb,
                        start=True,
                        stop=True,
                        is_transpose=True,
                    )
                if qb not in ot_alls:
                    ot_alls[qb] = stream.tile([128, 2, QB], bf16,
                                              tag="otall", bufs=2,
                                              name=f"otall{qb}")
                nc.vector.tensor_copy(
                    out=ot_alls[qb][:, pair, :],
                    in_=trp.rearrange("p a b -> p (a b)"),
                )

            # outproj(qb0) fillers bind to qb1's pool steps: emitted two
            # steps before pool-kc p, the psO tag chain delays their matmuls
            # until poolS(p) is copied, so the Act eviction lands exactly in
            # pool step p's Act idle.
            for qm in range(8):
                for jh in range(2):
                    slot = qm * 2 + jh  # 0..15
                    add_fill(1, slot // 4, POOL_KCS[slot % 4] - 2,
                             lambda qm=qm, jh=jh: emit_final_half(
                                 0, qm, jh, evict_act=True))

            # ---- flat software-pipelined stream ----
            sts[0] = emit_S(0)
            sts[1] = emit_S(1)
            for g, (qb, h, kc) in enumerate(steps):
                if g + 2 < NSTEP:
                    sts[g + 2] = emit_S(g + 2)
                emit_exp(g)
                for src in pv_due[g]:
                    emit_PV(src)
                # head completion: when the last PV emission position for
                # (qb', h') was at this g, normalize (and transpose on pairs)
                for key, (lg, lkc, lj) in list(bank_last.items()):
                    pqb, ph, pbk = key
                    if pbk == 1 and lg == g:
                        emit_norm(pqb, ph)
                        if ph % 2 == 1:
                            emit_transpose(pqb, ph // 2)
                for fn in fill.pop((qb, h, kc), ()):
                    fn()

            # ---- drain ----
            for g in range(NSTEP, NSTEP + PV_LEAD_POOL + 1):
                for src in pv_due[g]:
                    emit_PV(src)
                for key, (lg, lkc, lj) in list(bank_last.items()):
                    pqb, ph, pbk = key
                    if pbk == 1 and lg == g:
                        emit_norm(pqb, ph)
                        if ph % 2 == 1:
                            emit_transpose(pqb, ph // 2)

            if _DEBUG:
                nc.sync.dma_start(out=dbg_kt[:, :, :], in_=kT_sb)
                nc.sync.dma_start(out=dbg_qt[:, :, :], in_=qT0_sb)
                nc.sync.dma_start(out=dbg_v[:, :, :, :], in_=v_sb)
                nc.sync.dma_start(out=dbg_ot[:, :, :], in_=ot_alls[0])
                nc.sync.dma_start(out=dbg_pt[:, :], in_=dbg_pt_tile["t"])

            # tail: qb1 output projection round-robins over every psum
            # bank that is free after the last normalize (psS x2, psPV x2,
            # psT, psO), evictions alternating DVE/Act, so the groups and
            # their DMAs pipeline ~6 deep.
            tail_pools = [(psS, "S"), (psPV, "pv0"), (psT, "T"),
                          (psS, "S"), (psPV, "pv1"), (psO, "O")]
            for i, (qm, jh) in enumerate(
                    (qm, jh) for qm in range(8) for jh in range(2)):
                pool, tag = tail_pools[i % len(tail_pools)]
                emit_final_half(1, qm, jh, pool=pool, tag=tag,
                                evict_act=(i % 2 == 1))

    nc.finalize()
    return nc


def _get_nc():
    global _CACHED_NC
    if _CACHED_NC is None:
        _CACHED_NC = _build_bass()
    return _CACHED_NC


def _numpy_fallback(x, context, mask, Wq, Wk, Wv, Wout, bout):
    q = (x @ Wq.T).reshape(B, N, H, D)
    k = (context @ Wk.T).reshape(B, M, H, D)
    v = (context @ Wv.T).reshape(B, M, H, D)
    sim = np.einsum("bnhd,bmhd->bhnm", q, k) * SCALE
    sim = np.where(mask[:, None, None, :], sim, -np.finfo(np.float32).max)
    sim -= sim.max(axis=-1, keepdims=True)
    attn = np.exp(sim)
    attn /= attn.sum(axis=-1, keepdims=True)
    out = np.einsum("bhnm,bmhd->bnhd", attn, v).reshape(B, N, INNER)
    return (out @ Wout.T + bout).astype(np.float32)


def kernel(x, context, mask, Wq, Wk, Wv, Wout, bout, _want_results=False):
    import ml_dtypes

    bf = ml_dtypes.bfloat16
    x = np.asarray(x, dtype=np.float32)
    context = np.asarray(context, dtype=np.float32)
    mask = np.asarray(mask)
    Wq = np.asarray(Wq, dtype=np.float32)
    Wk = np.asarray(Wk, dtype=np.float32)
    Wv = np.asarray(Wv, dtype=np.float32)
    Wout = np.asarray(Wout, dtype=np.float32)
    bout = np.asarray(bout, dtype=np.float32)

    if not mask.all():
        return _numpy_fallback(x, context, mask, Wq, Wk, Wv, Wout, bout)

    from concourse.bass_utils import run_bass_kernel_spmd

    f8 = ml_dtypes.float8_e4m3

    def hilo(a, npairs):
        """fp8 hi/lo split in the DoubleRow pair layout [128, npairs, 2, n]."""
        hi = a.astype(f8)
        lo = (a - hi.astype(np.float32)).astype(f8)
        kp = a.shape[0] // 128

        def arr(v):
            return np.ascontiguousarray(
                v.reshape(npairs, 2, 128, a.shape[1]).transpose(2, 0, 1, 3))
        return arr(hi), arr(lo)

    ident = np.eye(128, dtype=np.float32).astype(bf)
    in_maps = []
    for c in range(N_CORES):
        b, hg = c // 2, c % 2
        sl = slice(hg * IN_PER_CORE, (hg + 1) * IN_PER_CORE)
        x8, xr = hilo(np.ascontiguousarray(x[b].T), 4)
        c8, cr = hilo(np.ascontiguousarray(context[b].T), 3)
        wq8, wqr = hilo(np.ascontiguousarray(Wq[sl, :].T) * W_SCALE, 4)
        wk8, wkr = hilo(np.ascontiguousarray(Wk[sl, :].T) * W_SCALE, 3)
        wv8, wvr = hilo(np.ascontiguousarray(Wv[sl, :].T) * W_SCALE, 3)
        in_maps.append(
            {
                "x8": x8, "xr": xr, "c8": c8, "cr": cr,
                "wq8": wq8, "wqr": wqr, "wk8": wk8, "wkr": wkr,
                "wv8": wv8, "wvr": wvr,
                "wo": np.ascontiguousarray(
                    Wout[:, sl].T.reshape(2, 128, 1024).transpose(1, 0, 2)
                ).astype(bf),
                "ident": ident,
            }
        )

    res = run_bass_kernel_spmd(_get_nc(), in_maps, core_ids=list(range(N_CORES)))

    out = np.empty((B, N, Q_DIM), dtype=np.float32)
    for b in range(B):
        out[b] = (res.results[2 * b]["out"].astype(np.float32)
                  + res.results[2 * b + 1]["out"].astype(np.float32) + bout)
    if _want_results:
        return out, res
    return out


# revision 20
# speedup vs baseline: 1.2391x; 1.0453x over previous
"""Trainium2 Bass kernel for CrossAttention (B=4, N=M=2048, H=8, D=64,
Q_DIM=1024, C_DIM=768).  v2: bf16 datapath + q-partitioned PV + split exp.

Sharding over 8 cores: core c handles batch b = c//2 and head-group
hg = c%2 (4 heads, 256 inner dims).  Each core computes a *partial*
output projection; the host sums core pairs and adds the output bias.

Key structure (chosen against the concourse TimelineSim cost model):
  - all matmul operands bf16 (1 cycle/row at any width); accumulation f32.
  - scores S.T[keys, q] per (qb, h, kc): 2 ap-512 matmuls (K=64).
  - PV is q-partitioned: out[128q, 65] per q-chunk with pt chunk stationary
    and v (with an appended ones-column -> softmax denominators) moving:
    520 cols per kc step instead of 1024 -> half the PE cost of the
    keys-partitioned form.  The 8 q-chunk accumulation groups share two
    PSUM banks via a single start/stop per bank (start marks the whole
    2KB zero region; first touch of each chunk overwrites).
  - exp is split: ~2/3 of score tiles on Act (activation Exp), ~1/3 via
    DVE copy to SBUF + GPSIMD pow(e^SCALE, S) (GPSIMD cannot read PSUM).
    Pool-path PV consumption is deferred 3 steps so the PE never waits.
  - normalization folds into the PV eviction: DVE reciprocal of the
    denominator column, then tensor_scalar_mul into packed bf16 tiles.
  - packed [128q, 128inner] head-pair tiles are PE-transposed (identity
    permutation rhs) so the output projection gets inner-contracted lhsT.
  - output projection per 128-query chunk: 2x2 ap-512 matmuls, evicted to
    SBUF and DMA'd per chunk.

The attention mask in this problem is all-True; if a mask with False
entries is ever passed, kernel() falls back to a numpy reference.
"""

import numpy as np

B, N, M = 4, 2048, 2048
Q_DIM, C_DIM, H, D = 1024, 768, 8, 64
INNER = H * D  # 512
SCALE = D ** -0.5

N_CORES = 8
W_SCALE = 32.0  # fp8 weight pre-scale so residuals clear the e4m3 subnormal floor
H_PER_CORE = 4
IN_PER_CORE = H_PER_CORE * D  # 256
QB = 1024
N_QB = N // QB          # 2
KC = M // 128           # 16 key chunks
QK_CHUNKS = Q_DIM // 128   # 8
CK_CHUNKS = C_DIM // 128   # 6

# key-chunks whose exp runs on GPSIMD (via DVE psum->sbuf copy); the rest
# run on Act.  Pool-path score tiles live OUTSIDE the 2-deep psS ring (one
# 512 half each in psT/psO) so the Act exp chain never waits on the copy;
# their PV consumption is deferred 4 steps.  The two DMA-paced early heads
# stay all-Act.
POOL_KCS = (3, 6, 9, 11)
NO_POOL_HEADS = ((0, 0), (0, 1))
PV_LEAD_ACT = 1
PV_LEAD_POOL = 4

_CACHED_NC = None
_DEBUG = False


def _build_bass():
    import concourse.bass as bass
    import concourse.mybir as mybir
    import concourse.tile as tile
    from concourse import bacc

    f32 = mybir.dt.float32
    f32r = mybir.dt.float32r
    bf16 = mybir.dt.bfloat16
    ts, ds = bass.ts, bass.ds
    Exp = mybir.ActivationFunctionType.Exp
    Pow = mybir.AluOpType.pow

    nc = bacc.Bacc("TRN2", target_bir_lowering=False)

    # all projection inputs arrive as fp8 hi/lo pairs, pre-arranged in
    # their DoubleRow SBUF layouts (contraction-chunk pairs on a free dim)
    fp8 = mybir.dt.float8e4
    x8_d = nc.dram_tensor("x8", [128, 4, 2, N], fp8, kind="ExternalInput")
    xr_d = nc.dram_tensor("xr", [128, 4, 2, N], fp8, kind="ExternalInput")
    c8_d = nc.dram_tensor("c8", [128, 3, 2, M], fp8, kind="ExternalInput")
    cr_d = nc.dram_tensor("cr", [128, 3, 2, M], fp8, kind="ExternalInput")
    wq8_d = nc.dram_tensor("wq8", [128, 4, 2, IN_PER_CORE], fp8, kind="ExternalInput")
    wqr_d = nc.dram_tensor("wqr", [128, 4, 2, IN_PER_CORE], fp8, kind="ExternalInput")
    wk8_d = nc.dram_tensor("wk8", [128, 3, 2, IN_PER_CORE], fp8, kind="ExternalInput")
    wkr_d = nc.dram_tensor("wkr", [128, 3, 2, IN_PER_CORE], fp8, kind="ExternalInput")
    wv8_d = nc.dram_tensor("wv8", [128, 3, 2, IN_PER_CORE], fp8, kind="ExternalInput")
    wvr_d = nc.dram_tensor("wvr", [128, 3, 2, IN_PER_CORE], fp8, kind="ExternalInput")
    wo = nc.dram_tensor("wo", [128, 2, Q_DIM], bf16, kind="ExternalInput")
    ident_d = nc.dram_tensor("ident", [128, 128], bf16, kind="ExternalInput")
    out_d = nc.dram_tensor("out", [N, Q_DIM], bf16, kind="ExternalOutput")
    if _DEBUG:
        dbg_kt = nc.dram_tensor("dbg_kt", [128, 2, M], bf16, kind="ExternalOutput")
        dbg_qt = nc.dram_tensor("dbg_qt", [128, 2, QB], bf16, kind="ExternalOutput")
        dbg_v = nc.dram_tensor("dbg_v", [128, KC, H_PER_CORE, D + 1], bf16, kind="ExternalOutput")
        dbg_ot = nc.dram_tensor("dbg_ot", [128, 2, QB], bf16, kind="ExternalOutput")
        dbg_pt = nc.dram_tensor("dbg_pt", [128, QB], bf16, kind="ExternalOutput")

    with tile.TileContext(nc) as tc:
        with (
            tc.tile_pool(name="persist", bufs=1) as persist,
            tc.tile_pool(name="stream", bufs=2) as stream,
            tc.tile_pool(name="psS", bufs=2, space="PSUM") as psS,
            tc.tile_pool(name="psPV", bufs=1, space="PSUM") as psPV,
            tc.tile_pool(name="psT", bufs=1, space="PSUM") as psT,
            tc.tile_pool(name="psO", bufs=1, space="PSUM") as psO,
        ):
            # ---- constants ----
            onesb = persist.tile([128, 64], bf16, tag="onesb")
            nc.vector.memset(onesb, 1.0)
            base = persist.tile([128, QB], f32, tag="base")
            nc.vector.memset(base, float(np.exp(SCALE)))
            wrm = persist.tile([128, 512], bf16, tag="wrm")
            nc.vector.memset(wrm, 0.0)
            # warm the Act exp table during the DMA shadow
            warm2 = persist.tile([128, 1], f32, tag="warm2")
            nc.scalar.activation(out=warm2, in_=wrm[:, 0:1], func=Exp,
                                 scale=SCALE)

            # PE p-state warmup: keep the tensor engine continuously busy
            # through the initial DMA wait so the first real matmuls run at
            # full clock (the cost model ramps 0.65->1.2->2.4 GHz over 3us
            # of continuous execution).
            warm_ps = psO.tile([128, 512], f32, tag="O", name="warmps")
            N_WARM = 13
            for i in range(N_WARM):
                nc.tensor.matmul(warm_ps, wrm[:, 0:128], wrm,
                                 start=(i == 0), stop=(i == N_WARM - 1))

            # ---- DMA schedule (execution order == emission order) ----
            # qT chain first (wq + x0-j0), then kT chain (wk + ctx-q0).
            ctx_sb = persist.tile([128, 2, 3, 2, M], fp8, tag="ctx")

            def dma_ctx_quarter(q):
                nc.sync.dma_start(
                    out=ctx_sb[:, 0, :, :, ds(q * 512, 512)],
                    in_=c8_d[:, :, :, ds(q * 512, 512)],
                )
                nc.sync.dma_start(
                    out=ctx_sb[:, 1, :, :, ds(q * 512, 512)],
                    in_=cr_d[:, :, :, ds(q * 512, 512)],
                )

            wq_sb = persist.tile([128, 2, 4, 2, IN_PER_CORE], fp8, tag="wq")
            nc.sync.dma_start(out=wq_sb[:, 0], in_=wq8_d[:, :, :, :])

            # x0 split by j-half so qT(qb0, m0, j0) completes early.
            # hl dim: 0 = fp8 high part, 1 = fp8 residual.
            x0_sb = stream.tile([128, 2, 4, 2, QB], fp8, tag="x0", bufs=1,
                                name="x0")
            nc.sync.dma_start(out=x0_sb[:, 0, :, :, 0:512],
                              in_=x8_d[:, :, :, 0:512])
            nc.sync.dma_start(out=wq_sb[:, 1], in_=wqr_d[:, :, :, :])
            nc.sync.dma_start(out=x0_sb[:, 1, :, :, 0:512],
                              in_=xr_d[:, :, :, 0:512])

            wk_sb = persist.tile([128, 2, 3, 2, IN_PER_CORE], fp8, tag="wk")
            nc.sync.dma_start(out=wk_sb[:, 0], in_=wk8_d[:, :, :, :])
            nc.sync.dma_start(out=wk_sb[:, 1], in_=wkr_d[:, :, :, :])

            dma_ctx_quarter(0)

            nc.sync.dma_start(out=x0_sb[:, 0, :, :, 512:1024],
                              in_=x8_d[:, :, :, 512:1024])
            nc.sync.dma_start(out=x0_sb[:, 1, :, :, 512:1024],
                              in_=xr_d[:, :, :, 512:1024])

            wv_sb = persist.tile([128, 2, 3, 2, IN_PER_CORE], fp8, tag="wv")
            nc.sync.dma_start(out=wv_sb[:, 0], in_=wv8_d[:, :, :, :])
            nc.sync.dma_start(out=wv_sb[:, 1], in_=wvr_d[:, :, :, :])

            dma_ctx_quarter(1)
            dma_ctx_quarter(2)
            dma_ctx_quarter(3)

            x1_sb = stream.tile([128, 2, 4, 2, QB], fp8, tag="x1", bufs=1,
                                name="x1")
            nc.sync.dma_start(out=x1_sb[:, 0], in_=x8_d[:, :, :, QB : 2 * QB])
            nc.sync.dma_start(out=x1_sb[:, 1], in_=xr_d[:, :, :, QB : 2 * QB])

            ident_sb = persist.tile([128, 128], bf16, tag="ident")
            nc.sync.dma_start(out=ident_sb, in_=ident_d[:, :])

            wo_sb = persist.tile([128, 2, Q_DIM], bf16, tag="wo")
            nc.sync.dma_start(out=wo_sb, in_=wo[:, :, :])

            # ---- persistent compute targets ----
            kT_sb = persist.tile([128, 2, M], bf16, tag="kt")
            v_sb = persist.tile([128, KC, H_PER_CORE, D + 1], bf16, tag="v")
            nc.vector.tensor_copy(
                out=v_sb[:, :, :, D : D + 1],
                in_=onesb.rearrange("p (a b c) -> p a b c", a=KC, b=H_PER_CORE),
            )

            # ---- projection pieces (3-term fp8 DoubleRow: hi*hi, hi*lo,
            # lo*hi; the lo*lo term is ~1e-6 relative and dropped) ----
            DR = mybir.MatmulPerfMode.DoubleRow
            TERMS = ((0, 0), (0, 1), (1, 0))

            def emit_kt_q(m, q, pool, tag):
                """kT_sb[:, m, q*512:(q+1)*512]: 9 DR matmuls."""
                kq = pool.tile([128, 512], f32, tag=tag, name=f"kq{m}{q}")
                n = 0
                for tw, tx in TERMS:
                    for pr in range(3):
                        n += 1
                        nc.tensor.matmul(
                            kq,
                            wk_sb[:, tw, pr, :, ts(m, 128)],
                            ctx_sb[:, tx, pr, :, ds(q * 512, 512)],
                            start=(n == 1),
                            stop=(n == 9),
                            perf_mode=DR,
                        )
                nc.vector.tensor_scalar_mul(kT_sb[:, m, ts(q, 512)], kq,
                                             1.0 / W_SCALE)

            _vps_open = {}

            def emit_v_chunk(kc, part=None):
                """part=None: whole chunk; part=0/1: half the contraction
                (the psum tile stays open between the two halves, so no
                other psT user may be emitted in between)."""
                if part in (None, 0):
                    # padded to 2KB so every psT tile shares one size class
                    _vps_open[kc] = psT.tile([128, 512], f32, tag="T",
                                             name=f"vps{kc}")
                vps = _vps_open[kc]
                lo, hi = {None: (0, 9), 0: (0, 5), 1: (5, 9)}[part]
                pieces = [(tw, tx, pr) for tw, tx in TERMS for pr in range(3)]
                for n in range(lo, hi):
                    tw, tx, pr = pieces[n]
                    nc.tensor.matmul(
                        vps[:, 0:IN_PER_CORE],
                        ctx_sb[:, tx, pr, :, ts(kc, 128)],
                        wv_sb[:, tw, pr, :, :],
                        start=(n == 0),
                        stop=(n == 8),
                        perf_mode=DR,
                    )
                if part in (None, 1):
                    nc.vector.tensor_scalar_mul(
                        v_sb[:, kc, :, 0:D],
                        _vps_open.pop(kc)[:, 0:IN_PER_CORE].rearrange(
                            "p (h d) -> p h d", h=H_PER_CORE),
                        1.0 / W_SCALE,
                    )

            # atomic qT j-half (8 matmuls + evict; single-buffer pools need
            # each user to fully retire before the next opens)
            def emit_qt_j(qT_sb, x_sb, m, qb, jh, pool=None, tag=None):
                pool = pool or psO
                tag = tag or "O"
                qps = pool.tile([128, 512], f32, tag=tag,
                                name=f"qj{qb}{m}{jh}")
                n = 0
                for tw, tx in TERMS:
                    for pr in range(4):
                        n += 1
                        nc.tensor.matmul(
                            qps,
                            wq_sb[:, tw, pr, :, ts(m, 128)],
                            x_sb[:, tx, pr, :, ds(jh * 512, 512)],
                            start=(n == 1),
                            stop=(n == 12),
                            perf_mode=DR,
                        )
                nc.vector.tensor_scalar_mul(qT_sb[:, m, ts(jh, 512)], qps,
                                             1.0 / W_SCALE)

            # ---- prologue (PE order matches DMA arrival order).  The
            # first two score tiles are emitted per j-half so their j0
            # exps can run while x0-j1 is still in flight. ----
            qT0_sb = stream.tile([128, 2, QB], bf16, tag="qt0", bufs=1,
                                 name="qT0")
            emit_qt_j(qT0_sb, x0_sb, 0, 0, 0, pool=psS, tag="S")
            emit_kt_q(0, 0, psS, "S")
            # early half-score tiles in psT/psO (separate tiles per half so
            # the j0 exps carry no false dependency on the j1 writes)
            early_half = {}
            for g0 in (0, 1):
                early_half[(g0, 0)] = psT.tile([128, 512], f32, tag="T",
                                               name=f"se{g0}a")
                nc.tensor.matmul(
                    early_half[(g0, 0)],
                    kT_sb[0:D, 0, ts(g0, 128)],
                    qT0_sb[0:D, 0, 0:512],
                    start=True, stop=True,
                )
            emit_v_chunk(0)
            emit_qt_j(qT0_sb, x0_sb, 0, 0, 1)
            for g0 in (0, 1):
                early_half[(g0, 1)] = psO.tile([128, 512], f32, tag="O",
                                               name=f"se{g0}b")
                nc.tensor.matmul(
                    early_half[(g0, 1)],
                    kT_sb[0:D, 0, ts(g0, 128)],
                    qT0_sb[0:D, 0, 512:1024],
                    start=True, stop=True,
                )
            emit_v_chunk(1)
            emit_v_chunk(2)
            emit_v_chunk(3)

            qT1_sb = stream.tile([128, 2, QB], bf16, tag="qt1", bufs=1,
                                 name="qT1")
            qT_tiles = [qT0_sb, qT1_sb]

            # ---- filler slot table: fill[(qb, h, kc)] -> list of fns ----
            fill = {}

            def add_fill(qb, h, kc, fn):
                fill.setdefault((qb, h, kc), []).append(fn)

            # Fillers are atomic (open + matmuls + evict) and spaced >=3
            # steps apart so the 2-deep score ring / Act exp queue absorbs
            # each clump before the next.  v chunks in h0 are split halves
            # (psT has no other user there).
            add_fill(0, 0, 0, lambda: emit_kt_q(0, 1, psO, "O"))
            add_fill(0, 0, 3, lambda: emit_kt_q(0, 2, psO, "O"))
            add_fill(0, 0, 6, lambda: emit_kt_q(0, 3, psO, "O"))
            for kc0, slot in ((4, 1), (5, 2), (6, 3), (7, 4), (8, 5), (9, 6),
                              (10, 7), (11, 8), (12, 9), (13, 10), (14, 11),
                              (15, 13)):
                add_fill(0, 0, slot, lambda kc=kc0: emit_v_chunk(kc, 0))
                add_fill(0, 0, slot + 1, lambda kc=kc0: emit_v_chunk(kc, 1))
            # kT(m1, q0..q3) gate S(h2, *): fill h1 slots (ctx all landed).
            add_fill(0, 1, 1, lambda: emit_kt_q(1, 0, psO, "O"))
            add_fill(0, 1, 3, lambda: emit_kt_q(1, 1, psO, "O"))
            add_fill(0, 1, 5, lambda: emit_kt_q(1, 2, psO, "O"))
            add_fill(0, 1, 7, lambda: emit_kt_q(1, 3, psO, "O"))
            # qT(qb0, m1): gates S(h2) (emitted step 30).
            add_fill(0, 1, 9, lambda: emit_qt_j(qT0_sb, x0_sb, 1, 0, 0))
            add_fill(0, 1, 11, lambda: emit_qt_j(qT0_sb, x0_sb, 1, 0, 1))
            # qT(qb1, m0): gates S(qb1, h0) (emitted step 62); x1 by ~28us.
            add_fill(0, 2, 2, lambda: emit_qt_j(qT1_sb, x1_sb, 0, 1, 0))
            add_fill(0, 2, 8, lambda: emit_qt_j(qT1_sb, x1_sb, 0, 1, 1))
            # qT(qb1, m1): gates S(qb1, h2) (emitted step 94): qb1-h0 slots.
            add_fill(1, 0, 11, lambda: emit_qt_j(qT1_sb, x1_sb, 1, 1, 0))
            add_fill(1, 0, 13, lambda: emit_qt_j(qT1_sb, x1_sb, 1, 1, 1))

            # ---- final projection: one (qm, jh) half ----
            ost_tiles = {}
            opool_toggle = {}

            def emit_final_half(qb, qm, jh, pool=None, tag=None,
                                evict_act=False):
                pool = pool or psO
                tag = tag or "O"
                ops = pool.tile([128, 512], f32, tag=tag,
                                name=f"ops{qb}{qm}{jh}")
                for t in range(2):
                    nc.tensor.matmul(
                        ops,
                        ot_alls[qb][:, t, ts(qm, 128)],
                        wo_sb[:, t, ts(jh, 512)],
                        start=(t == 0),
                        stop=(t == 1),
                    )
                if (qb, qm) not in ost_tiles:
                    ost_tiles[(qb, qm)] = stream.tile(
                        [128, Q_DIM], bf16, tag="ost", bufs=3,
                        name=f"ost{qb}{qm}"
                    )
                ost = ost_tiles[(qb, qm)]
                if evict_act:
                    nc.scalar.copy(out=ost[:, ts(jh, 512)], in_=ops)
                else:
                    nc.vector.tensor_copy(out=ost[:, ts(jh, 512)], in_=ops)
                if jh == 1:
                    nc.sync.dma_start(
                        out=out_d[ds(qb * QB + qm * 128, 128), :], in_=ost
                    )

            # ---- attention state ----
            ot_alls = {}
            pv_banks = {}    # (qb, h) -> [tile_a, tile_b]
            pk_tiles = {}    # (qb, pair) -> packed normalized tile
            pts = {}         # step -> pt tile
            sts = {}         # step -> score tile (act) or sbuf copy (pool)

            steps = [(qb, h, kc)
                     for qb in range(N_QB)
                     for h in range(H_PER_CORE)
                     for kc in range(KC)]
            NSTEP = len(steps)

            def is_pool(qb, h, kc):
                return kc in POOL_KCS and (qb, h) not in NO_POOL_HEADS

            def lead_of(qb, h, kc):
                if is_pool(qb, h, kc):
                    return PV_LEAD_POOL
                # stagger the first PVs of each head so they never wait on
                # the previous head's normalize reads (psPV is single-buffered)
                return {0: 3, 1: 2}.get(kc, PV_LEAD_ACT)

            # per-step PV due lists: pv_due[g] = list of source steps
            pv_due = [[] for _ in range(NSTEP + PV_LEAD_POOL + 1)]
            for g, (qb, h, kc) in enumerate(steps):
                pv_due[g + lead_of(qb, h, kc)].append(g)
            # per-(qb,h) bank emission bookkeeping: which (kc, j) is first/
            # last per bank in emission order.
            bank_seq = {}  # (qb, h, bank) -> [positions...] as (kc, j)
            for g, (qb, h, kc) in enumerate(steps):
                for j in range(8):
                    bank_seq.setdefault((qb, h, j // 4), []).append(
                        (g + lead_of(qb, h, kc), kc, j))
            for key in bank_seq:
                bank_seq[key].sort()
            bank_first = {k: v[0] for k, v in bank_seq.items()}
            bank_last = {k: v[-1] for k, v in bank_seq.items()}

            def emit_S(g):
                if g <= 1:
                    return None  # prologue-emitted half tiles
                qb, h, kc = steps[g]
                t, po = h // 2, (h % 2) * D
                if is_pool(qb, h, kc):
                    # pool-path scores live outside the psS ring: one 512
                    # half in psT, one in psO
                    st = (
                        psT.tile([128, 512], f32, tag="T", name=f"sp{qb}{h}{kc}a"),
                        psO.tile([128, 512], f32, tag="O", name=f"sp{qb}{h}{kc}b"),
                    )
                    for j in range(2):
                        nc.tensor.matmul(
                            st[j],
                            kT_sb[po : po + D, t, ts(kc, 128)],
                            qT_tiles[qb][po : po + D, t, ts(j, 512)],
                            start=True,
                            stop=True,
                        )
                    return st
                st = psS.tile([128, QB], f32, tag="S", name=f"st{qb}{h}{kc}")
                for j in range(2):
                    nc.tensor.matmul(
                        st[:, ts(j, 512)],
                        kT_sb[po : po + D, t, ts(kc, 128)],
                        qT_tiles[qb][po : po + D, t, ts(j, 512)],
                        start=True,
                        stop=True,
                    )
                return st

            dbg_pt_tile = {}

            def emit_exp(g):
                if g <= 1:
                    return  # emitted before the loop
                qb, h, kc = steps[g]
                pt = stream.tile([128, QB], bf16, tag="pt", bufs=6,
                                 name=f"pt{qb}{h}{kc}")
                if _DEBUG and g == 0:
                    dbg_pt_tile["t"] = stream.tile([128, QB], bf16, tag="dbgpt",
                                                   bufs=1, name="dbgpt")
                st = sts.pop(g)
                if g <= 1:
                    return  # handled by the early half-exp block
                if is_pool(qb, h, kc):
                    st_sb = stream.tile([128, QB], f32, tag="stsb", bufs=2,
                                        name=f"stsb{qb}{h}{kc}")
                    nc.vector.tensor_copy(out=st_sb[:, 0:512], in_=st[0])
                    nc.vector.tensor_copy(out=st_sb[:, 512:1024], in_=st[1])
                    nc.gpsimd.tensor_tensor(out=pt, in0=base, in1=st_sb,
                                            op=Pow)
                else:
                    nc.scalar.activation(out=pt, in_=st, func=Exp,
                                         scale=SCALE)
                if _DEBUG and g == 0:
                    nc.vector.tensor_copy(out=dbg_pt_tile["t"], in_=pt)
                pts[g] = pt

            def emit_PV(src_g):
                qb, h, kc = steps[src_g]
                if (qb, h) not in pv_banks:
                    pv_banks[(qb, h)] = [
                        psPV.tile([128, 4, D + 1], f32, tag=f"pv{bk}",
                                  name=f"pv{qb}{h}{bk}")
                        for bk in range(2)
                    ]
                pt = pts.pop(src_g)
                banks = pv_banks[(qb, h)]
                for j in range(8):
                    bk = j // 4
                    nc.tensor.matmul(
                        banks[bk][:, j % 4, :],
                        pt[:, ts(j, 128)],
                        v_sb[:, kc, h, :],
                        start=((kc, j) == bank_first[(qb, h, bk)][1:]),
                        stop=((kc, j) == bank_last[(qb, h, bk)][1:]),
                    )

            Copy = mybir.ActivationFunctionType.Copy

            _norm_state = {}

            def emit_norm(qb, h, burst=None):
                """recip + normalize-evict into the packed pair tile, split
                into two 4-mult bursts on consecutive steps so the DVE never
                sees one long burst.  In the tail Act helps (it is idle)."""
                tail = (qb, h) == (N_QB - 1, H_PER_CORE - 1)
                pair, slot = h // 2, h % 2
                if (qb, pair) not in pk_tiles:
                    pk_tiles[(qb, pair)] = stream.tile(
                        [128, 8, 2, D], bf16, tag="pk", bufs=2,
                        name=f"pk{qb}{pair}"
                    )
                pk = pk_tiles[(qb, pair)]
                if burst in (None, 0):
                    banks = pv_banks.pop((qb, h))
                    recip = stream.tile([128, 8], f32, tag="recip", bufs=2,
                                        name=f"rc{qb}{h}")
                    for bk in range(2):
                        nc.vector.reciprocal(out=recip[:, ds(bk * 4, 4)],
                                             in_=banks[bk][:, :, D])
                    _norm_state[(qb, h)] = (banks, recip)
                banks, recip = _norm_state[(qb, h)]
                jlo, jhi = {None: (0, 8), 0: (0, 4), 1: (4, 8)}[burst]
                for j in range(jlo, jhi):
                    if tail and j % 2 == 1:
                        nc.scalar.activation(
                            out=pk[:, j, slot, :],
                            in_=banks[j // 4][:, j % 4, 0:D],
                            func=Copy, scale=recip[:, ds(j, 1)],
                        )
                    else:
                        nc.vector.tensor_scalar_mul(
                            pk[:, j, slot, :],
                            banks[j // 4][:, j % 4, 0:D],
                            recip[:, ds(j, 1)],
                        )
                if burst == 1 or burst is None:
                    _norm_state.pop((qb, h))

            def emit_transpose(qb, pair):
                pk = pk_tiles.pop((qb, pair))
                trp = psT.tile([128, 8, 128], bf16, tag="T",
                               name=f"trp{qb}{pair}")
                for qc in range(8):
                    nc.tensor.matmul(
                        trp[:, qc, :],
                        pk[:, qc, :, :],
                        ident_sb,
                        start=True,
                        stop=True,
                        is_transpose=True,
                    )
                if qb not in ot_alls:
                    ot_alls[qb] = stream.tile([128, 2, QB], bf16,
                                              tag="otall", bufs=2,
                                              name=f"otall{qb}")
                if (qb, pair) == (N_QB - 1, 1):
                    # tail: split across DVE and the now-idle Act so the
                    # first output-projection groups start sooner
                    flat = trp.rearrange("p a b -> p (a b)")
                    nc.vector.tensor_copy(
                        out=ot_alls[qb][:, pair, 0:512], in_=flat[:, 0:512])
                    nc.scalar.copy(
                        out=ot_alls[qb][:, pair, 512:1024],
                        in_=flat[:, 512:1024])
                else:
                    nc.vector.tensor_copy(
                        out=ot_alls[qb][:, pair, :],
                        in_=trp.rearrange("p a b -> p (a b)"),
                    )

            # outproj(qb0) fillers bind to qb1's pool steps: emitted two
            # steps before pool-kc p, the psO tag chain delays their matmuls
            # until poolS(p) is copied, so the Act eviction lands exactly in
            # pool step p's Act idle.
            for qm in range(8):
                for jh in range(2):
                    slot = qm * 2 + jh  # 0..15
                    add_fill(1, slot // 4, POOL_KCS[slot % 4] - 2,
                             lambda qm=qm, jh=jh: emit_final_half(
                                 0, qm, jh, evict_act=False))

            pend = {}

            def add_pend(g, fn):
                pend.setdefault(g, []).append(fn)

            # ---- flat software-pipelined stream ----
            # steps 0/1: half-exps in readiness order (j0 halves first)
            for g0 in (0, 1):
                pts[g0] = stream.tile([128, QB], bf16, tag="pt", bufs=6,
                                      name=f"pte{g0}")
            for jh in (0, 1):
                for g0 in (0, 1):
                    nc.scalar.activation(out=pts[g0][:, ts(jh, 512)],
                                         in_=early_half.pop((g0, jh)),
                                         func=Exp, scale=SCALE)
            sts[0] = sts[1] = None
            for g, (qb, h, kc) in enumerate(steps):
                if g + 2 < NSTEP:
                    sts[g + 2] = emit_S(g + 2)
                emit_exp(g)
                for fn in pend.pop(g, ()):
                    fn()
                for src in pv_due[g]:
                    emit_PV(src)
                # head completion: when the last PV emission position for
                # (qb', h') was at this g, normalize (and transpose on pairs)
                for key, (lg, lkc, lj) in list(bank_last.items()):
                    pqb, ph, pbk = key
                    if pbk == 1 and lg == g:
                        if (pqb, ph) == (N_QB - 1, H_PER_CORE - 1):
                            emit_norm(pqb, ph)
                            emit_transpose(pqb, 1)
                        else:
                            emit_norm(pqb, ph, burst=0)
                            add_pend(g + 1, lambda pqb=pqb, ph=ph:
                                     emit_norm(pqb, ph, burst=1))
                            if ph % 2 == 1:
                                add_pend(g + 4, lambda pqb=pqb, ph=ph:
                                         emit_transpose(pqb, ph // 2))
                for fn in fill.pop((qb, h, kc), ()):
                    fn()

            # ---- drain ----
            for g in range(NSTEP, NSTEP + PV_LEAD_POOL + 1):
                for fn in pend.pop(g, ()):
                    fn()
                for src in pv_due[g]:
                    emit_PV(src)
                for key, (lg, lkc, lj) in list(bank_last.items()):
                    pqb, ph, pbk = key
                    if pbk == 1 and lg == g:
                        emit_norm(pqb, ph)
                        if ph % 2 == 1:
                            emit_transpose(pqb, ph // 2)
            for g in sorted(pend):
                for fn in pend[g]:
                    fn()

            if _DEBUG:
                nc.sync.dma_start(out=dbg_kt[:, :, :], in_=kT_sb)
                nc.sync.dma_start(out=dbg_qt[:, :, :], in_=qT0_sb)
                nc.sync.dma_start(out=dbg_v[:, :, :, :], in_=v_sb)
                nc.sync.dma_start(out=dbg_ot[:, :, :], in_=ot_alls[0])
                nc.sync.dma_start(out=dbg_pt[:, :], in_=dbg_pt_tile["t"])

            # tail: qb1 output projection round-robins over every psum
            # bank that is free after the last normalize (psS x2, psPV x2,
            # psT, psO), evictions alternating DVE/Act, so the groups and
            # their DMAs pipeline ~6 deep.
            tail_pools = [(psS, "S"), (psPV, "pv0"), (psT, "T"),
                          (psS, "S"), (psPV, "pv1"), (psO, "O")]
            for i, (qm, jh) in enumerate(
                    (qm, jh) for qm in range(8) for jh in range(2)):
                pool, tag = tail_pools[i % len(tail_pools)]
                emit_final_half(1, qm, jh, pool=pool, tag=tag,
                                evict_act=(i % 2 == 1))

    nc.finalize()
    return nc


def _get_nc():
    global _CACHED_NC
    if _CACHED_NC is None:
        _CACHED_NC = _build_bass()
    return _CACHED_NC


def _numpy_fallback(x, context, mask, Wq, Wk, Wv, Wout, bout):
    q = (x @ Wq.T).reshape(B, N, H, D)
    k = (context @ Wk.T).reshape(B, M, H, D)
    v = (context @ Wv.T).reshape(B, M, H, D)
    sim = np.einsum("bnhd,bmhd->bhnm", q, k) * SCALE
    sim = np.where(mask[:, None, None, :], sim, -np.finfo(np.float32).max)
    sim -= sim.max(axis=-1, keepdims=True)
    attn = np.exp(sim)
    attn /= attn.sum(axis=-1, keepdims=True)
    out = np.einsum("bhnm,bmhd->bnhd", attn, v).reshape(B, N, INNER)
    return (out @ Wout.T + bout).astype(np.float32)


def kernel(x, context, mask, Wq, Wk, Wv, Wout, bout, _want_results=False):
    import ml_dtypes

    bf = ml_dtypes.bfloat16
    x = np.asarray(x, dtype=np.float32)
    context = np.asarray(context, dtype=np.float32)
    mask = np.asarray(mask)
    Wq = np.asarray(Wq, dtype=np.float32)
    Wk = np.asarray(Wk, dtype=np.float32)
    Wv = np.asarray(Wv, dtype=np.float32)
    Wout = np.asarray(Wout, dtype=np.float32)
    bout = np.asarray(bout, dtype=np.float32)

    if not mask.all():
        return _numpy_fallback(x, context, mask, Wq, Wk, Wv, Wout, bout)

    from concourse.bass_utils import run_bass_kernel_spmd

    f8 = ml_dtypes.float8_e4m3

    def hilo(a, npairs):
        """fp8 hi/lo split in the DoubleRow pair layout [128, npairs, 2, n]."""
        hi = a.astype(f8)
        lo = (a - hi.astype(np.float32)).astype(f8)
        kp = a.shape[0] // 128

        def arr(v):
            return np.ascontiguousarray(
                v.reshape(npairs, 2, 128, a.shape[1]).transpose(2, 0, 1, 3))
        return arr(hi), arr(lo)

    ident = np.eye(128, dtype=np.float32).astype(bf)
    in_maps = []
    for c in range(N_CORES):
        b, hg = c // 2, c % 2
        sl = slice(hg * IN_PER_CORE, (hg + 1) * IN_PER_CORE)
        x8, xr = hilo(np.ascontiguousarray(x[b].T), 4)
        c8, cr = hilo(np.ascontiguousarray(context[b].T), 3)
        wq8, wqr = hilo(np.ascontiguousarray(Wq[sl, :].T) * W_SCALE, 4)
        wk8, wkr = hilo(np.ascontiguousarray(Wk[sl, :].T) * W_SCALE, 3)
        wv8, wvr = hilo(np.ascontiguousarray(Wv[sl, :].T) * W_SCALE, 3)
        in_maps.append(
            {
                "x8": x8, "xr": xr, "c8": c8, "cr": cr,
                "wq8": wq8, "wqr": wqr, "wk8": wk8, "wkr": wkr,
                "wv8": wv8, "wvr": wvr,
                "wo": np.ascontiguousarray(
                    Wout[:, sl].T.reshape(2, 128, 1024).transpose(1, 0, 2)
                ).astype(bf),
                "ident": ident,
            }
        )

    res = run_bass_kernel_spmd(_get_nc(), in_maps, core_ids=list(range(N_CORES)))

    out = np.empty((B, N, Q_DIM), dtype=np.float32)
    for b in range(B):
        out[b] = (res.results[2 * b]["out"].astype(np.float32)
                  + res.results[2 * b + 1]["out"].astype(np.float32) + bout)
    if _want_results:
        return out, res
    return out


# revision 38
# speedup vs baseline: 1.3119x; 1.0588x over previous
"""Trainium2 Bass kernel for CrossAttention (B=4, N=M=2048, H=8, D=64,
Q_DIM=1024, C_DIM=768).  v2: bf16 datapath + q-partitioned PV + split exp.

Sharding over 8 cores: core c handles batch b = c//2 and head-group
hg = c%2 (4 heads, 256 inner dims).  Each core computes a *partial*
output projection; the host sums core pairs and adds the output bias.

Key structure (chosen against the concourse TimelineSim cost model):
  - all matmul operands bf16 (1 cycle/row at any width); accumulation f32.
  - scores S.T[keys, q] per (qb, h, kc): 2 ap-512 matmuls (K=64).
  - PV is q-partitioned: out[128q, 65] per q-chunk with pt chunk stationary
    and v (with an appended ones-column -> softmax denominators) moving:
    520 cols per kc step instead of 1024 -> half the PE cost of the
    keys-partitioned form.  The 8 q-chunk accumulation groups share two
    PSUM banks via a single start/stop per bank (start marks the whole
    2KB zero region; first touch of each chunk overwrites).
  - exp is split: ~2/3 of score tiles on Act (activation Exp), ~1/3 via
    DVE copy to SBUF + GPSIMD pow(e^SCALE, S) (GPSIMD cannot read PSUM).
    Pool-path PV consumption is deferred 3 steps so the PE never waits.
  - normalization folds into the PV eviction: DVE reciprocal of the
    denominator column, then tensor_scalar_mul into packed bf16 tiles.
  - packed [128q, 128inner] head-pair tiles are PE-transposed (identity
    permutation rhs) so the output projection gets inner-contracted lhsT.
  - output projection per 128-query chunk: 2x2 ap-512 matmuls, evicted to
    SBUF and DMA'd per chunk.

The attention mask in this problem is all-True; if a mask with False
entries is ever passed, kernel() falls back to a numpy reference.
"""

import numpy as np

B, N, M = 4, 2048, 2048
Q_DIM, C_DIM, H, D = 1024, 768, 8, 64
INNER = H * D  # 512
SCALE = D ** -0.5

N_CORES = 8
W_SCALE = 32.0  # fp8 weight pre-scale so residuals clear the e4m3 subnormal floor
H_PER_CORE = 4
IN_PER_CORE = H_PER_CORE * D  # 256
QB = 1024
N_QB = N // QB          # 2
KC = M // 128           # 16 key chunks
QK_CHUNKS = Q_DIM // 128   # 8
CK_CHUNKS = C_DIM // 128   # 6

# key-chunks whose exp runs on GPSIMD (via DVE psum->sbuf copy); the rest
# run on Act.  Pool-path score tiles live OUTSIDE the 2-deep psS ring (one
# 512 half each in psT/psO) so the Act exp chain never waits on the copy;
# their PV consumption is deferred 4 steps.  The two DMA-paced early heads
# stay all-Act.
POOL_KCS = (3, 6, 9, 11)
NO_POOL_HEADS = ((0, 0), (0, 1))
PV_LEAD_ACT = 1
PV_LEAD_POOL = 4

_CACHED_NC = None
_DEBUG = False


def _build_bass():
    import concourse.bass as bass
    import concourse.mybir as mybir
    import concourse.tile as tile
    from concourse import bacc

    f32 = mybir.dt.float32
    f32r = mybir.dt.float32r
    bf16 = mybir.dt.bfloat16
    ts, ds = bass.ts, bass.ds
    Exp = mybir.ActivationFunctionType.Exp
    Pow = mybir.AluOpType.pow

    nc = bacc.Bacc("TRN2", target_bir_lowering=False)

    # all projection inputs arrive as fp8 hi/lo pairs, pre-arranged in
    # their DoubleRow SBUF layouts (contraction-chunk pairs on a free dim)
    fp8 = mybir.dt.float8e4
    x8_d = nc.dram_tensor("x8", [128, 4, 2, N], fp8, kind="ExternalInput")
    xr_d = nc.dram_tensor("xr", [128, 4, 2, N], fp8, kind="ExternalInput")
    c8_d = nc.dram_tensor("c8", [128, 3, 2, M], fp8, kind="ExternalInput")
    cr_d = nc.dram_tensor("cr", [128, 3, 2, M], fp8, kind="ExternalInput")
    wq8_d = nc.dram_tensor("wq8", [128, 4, 2, IN_PER_CORE], fp8, kind="ExternalInput")
    wqr_d = nc.dram_tensor("wqr", [128, 4, 2, IN_PER_CORE], fp8, kind="ExternalInput")
    wk8_d = nc.dram_tensor("wk8", [128, 3, 2, IN_PER_CORE], fp8, kind="ExternalInput")
    wkr_d = nc.dram_tensor("wkr", [128, 3, 2, IN_PER_CORE], fp8, kind="ExternalInput")
    wv8_d = nc.dram_tensor("wv8", [128, 3, 2, IN_PER_CORE], fp8, kind="ExternalInput")
    wvr_d = nc.dram_tensor("wvr", [128, 3, 2, IN_PER_CORE], fp8, kind="ExternalInput")
    wo = nc.dram_tensor("wo", [128, 2, Q_DIM], bf16, kind="ExternalInput")
    ident_d = nc.dram_tensor("ident", [128, 128], bf16, kind="ExternalInput")
    out_d = nc.dram_tensor("out", [N, Q_DIM], bf16, kind="ExternalOutput")
    if _DEBUG:
        dbg_kt = nc.dram_tensor("dbg_kt", [128, 2, M], bf16, kind="ExternalOutput")
        dbg_qt = nc.dram_tensor("dbg_qt", [128, 2, QB], bf16, kind="ExternalOutput")
        dbg_v = nc.dram_tensor("dbg_v", [128, KC, H_PER_CORE, D + 1], bf16, kind="ExternalOutput")
        dbg_ot = nc.dram_tensor("dbg_ot", [128, 2, QB], bf16, kind="ExternalOutput")
        dbg_pt = nc.dram_tensor("dbg_pt", [128, QB], bf16, kind="ExternalOutput")

    with tile.TileContext(nc) as tc:
        with (
            tc.tile_pool(name="persist", bufs=1) as persist,
            tc.tile_pool(name="stream", bufs=2) as stream,
            tc.tile_pool(name="psS", bufs=2, space="PSUM") as psS,
            tc.tile_pool(name="psPV", bufs=1, space="PSUM") as psPV,
            tc.tile_pool(name="psT", bufs=1, space="PSUM") as psT,
            tc.tile_pool(name="psO", bufs=1, space="PSUM") as psO,
        ):
            # ---- constants ----
            onesb = persist.tile([128, 64], bf16, tag="onesb")
            nc.vector.memset(onesb, 1.0)
            base = persist.tile([128, QB], f32, tag="base")
            nc.vector.memset(base, float(np.exp(SCALE)))
            wrm = persist.tile([128, 512], bf16, tag="wrm")
            nc.vector.memset(wrm, 0.0)
            # warm the Act exp table during the DMA shadow
            warm2 = persist.tile([128, 1], f32, tag="warm2")
            nc.scalar.activation(out=warm2, in_=wrm[:, 0:1], func=Exp,
                                 scale=SCALE)

            # PE p-state warmup: keep the tensor engine continuously busy
            # through the initial DMA wait so the first real matmuls run at
            # full clock (the cost model ramps 0.65->1.2->2.4 GHz over 3us
            # of continuous execution).
            warm_ps = psO.tile([128, 512], f32, tag="O", name="warmps")
            N_WARM = 10
            for i in range(N_WARM):
                nc.tensor.matmul(warm_ps, wrm[:, 0:128], wrm,
                                 start=(i == 0), stop=(i == N_WARM - 1))

            # ---- DMA schedule (execution order == emission order) ----
            # qT chain first (wq + x0-j0), then kT chain (wk + ctx-q0).
            ctx_sb = persist.tile([128, 2, 3, 2, M], fp8, tag="ctx")

            def dma_ctx_quarter(q):
                nc.sync.dma_start(
                    out=ctx_sb[:, 0, :, :, ds(q * 512, 512)],
                    in_=c8_d[:, :, :, ds(q * 512, 512)],
                )
                nc.sync.dma_start(
                    out=ctx_sb[:, 1, :, :, ds(q * 512, 512)],
                    in_=cr_d[:, :, :, ds(q * 512, 512)],
                )

            wq_sb = persist.tile([128, 2, 4, 2, IN_PER_CORE], fp8, tag="wq")
            nc.sync.dma_start(out=wq_sb[:, 0], in_=wq8_d[:, :, :, :])

            # x0 split by j-half so qT(qb0, m0, j0) completes early.
            # hl dim: 0 = fp8 high part, 1 = fp8 residual.
            x0_sb = stream.tile([128, 2, 4, 2, QB], fp8, tag="x0", bufs=1,
                                name="x0")
            nc.sync.dma_start(out=x0_sb[:, 0, :, :, 0:512],
                              in_=x8_d[:, :, :, 0:512])
            nc.sync.dma_start(out=wq_sb[:, 1], in_=wqr_d[:, :, :, :])
            nc.sync.dma_start(out=x0_sb[:, 1, :, :, 0:512],
                              in_=xr_d[:, :, :, 0:512])

            wk_sb = persist.tile([128, 2, 3, 2, IN_PER_CORE], fp8, tag="wk")
            nc.sync.dma_start(out=wk_sb[:, 0], in_=wk8_d[:, :, :, :])
            nc.sync.dma_start(out=wk_sb[:, 1], in_=wkr_d[:, :, :, :])

            dma_ctx_quarter(0)

            nc.sync.dma_start(out=x0_sb[:, 0, :, :, 512:1024],
                              in_=x8_d[:, :, :, 512:1024])
            nc.sync.dma_start(out=x0_sb[:, 1, :, :, 512:1024],
                              in_=xr_d[:, :, :, 512:1024])

            wv_sb = persist.tile([128, 2, 3, 2, IN_PER_CORE], fp8, tag="wv")
            nc.sync.dma_start(out=wv_sb[:, 0], in_=wv8_d[:, :, :, :])
            nc.sync.dma_start(out=wv_sb[:, 1], in_=wvr_d[:, :, :, :])

            dma_ctx_quarter(1)
            dma_ctx_quarter(2)
            dma_ctx_quarter(3)

            x1_sb = stream.tile([128, 2, 4, 2, QB], fp8, tag="x1", bufs=1,
                                name="x1")
            nc.sync.dma_start(out=x1_sb[:, 0], in_=x8_d[:, :, :, QB : 2 * QB])
            nc.sync.dma_start(out=x1_sb[:, 1], in_=xr_d[:, :, :, QB : 2 * QB])

            ident_sb = persist.tile([128, 128], bf16, tag="ident")
            nc.sync.dma_start(out=ident_sb, in_=ident_d[:, :])

            wo_sb = persist.tile([128, 2, Q_DIM], bf16, tag="wo")
            nc.sync.dma_start(out=wo_sb, in_=wo[:, :, :])

            # ---- persistent compute targets ----
            kT_sb = persist.tile([128, 2, M], bf16, tag="kt")
            v_sb = persist.tile([128, KC, H_PER_CORE, D + 1], bf16, tag="v")
            nc.vector.tensor_copy(
                out=v_sb[:, :, :, D : D + 1],
                in_=onesb.rearrange("p (a b c) -> p a b c", a=KC, b=H_PER_CORE),
            )

            # ---- projection pieces (3-term fp8 DoubleRow: hi*hi, hi*lo,
            # lo*hi; the lo*lo term is ~1e-6 relative and dropped) ----
            DR = mybir.MatmulPerfMode.DoubleRow
            TERMS = ((0, 0), (0, 1), (1, 0))

            def emit_kt_q(m, q, pool, tag):
                """kT_sb[:, m, q*512:(q+1)*512]: 9 DR matmuls."""
                kq = pool.tile([128, 512], f32, tag=tag, name=f"kq{m}{q}")
                n = 0
                for tw, tx in TERMS:
                    for pr in range(3):
                        n += 1
                        nc.tensor.matmul(
                            kq,
                            wk_sb[:, tw, pr, :, ts(m, 128)],
                            ctx_sb[:, tx, pr, :, ds(q * 512, 512)],
                            start=(n == 1),
                            stop=(n == 9),
                            perf_mode=DR,
                        )
                nc.vector.tensor_scalar_mul(kT_sb[:, m, ts(q, 512)], kq,
                                             1.0 / W_SCALE)

            _vps_open = {}

            def emit_v_chunk(kc, part=None):
                """part=None: whole chunk; part=0/1: half the contraction
                (the psum tile stays open between the two halves, so no
                other psT user may be emitted in between)."""
                if part in (None, 0):
                    # padded to 2KB so every psT tile shares one size class
                    _vps_open[kc] = psT.tile([128, 512], f32, tag="T",
                                             name=f"vps{kc}")
                vps = _vps_open[kc]
                lo, hi = {None: (0, 9), 0: (0, 5), 1: (5, 9)}[part]
                pieces = [(tw, tx, pr) for tw, tx in TERMS for pr in range(3)]
                for n in range(lo, hi):
                    tw, tx, pr = pieces[n]
                    nc.tensor.matmul(
                        vps[:, 0:IN_PER_CORE],
                        ctx_sb[:, tx, pr, :, ts(kc, 128)],
                        wv_sb[:, tw, pr, :, :],
                        start=(n == 0),
                        stop=(n == 8),
                        perf_mode=DR,
                    )
                if part in (None, 1):
                    nc.vector.tensor_scalar_mul(
                        v_sb[:, kc, :, 0:D],
                        _vps_open.pop(kc)[:, 0:IN_PER_CORE].rearrange(
                            "p (h d) -> p h d", h=H_PER_CORE),
                        1.0 / W_SCALE,
                    )

            # atomic qT j-half (8 matmuls + evict; single-buffer pools need
            # each user to fully retire before the next opens)
            def emit_qt_j(qT_sb, x_sb, m, qb, jh, pool=None, tag=None):
                pool = pool or psO
                tag = tag or "O"
                qps = pool.tile([128, 512], f32, tag=tag,
                                name=f"qj{qb}{m}{jh}")
                n = 0
                for tw, tx in TERMS:
                    for pr in range(4):
                        n += 1
                        nc.tensor.matmul(
                            qps,
                            wq_sb[:, tw, pr, :, ts(m, 128)],
                            x_sb[:, tx, pr, :, ds(jh * 512, 512)],
                            start=(n == 1),
                            stop=(n == 12),
                            perf_mode=DR,
                        )
                nc.vector.tensor_scalar_mul(qT_sb[:, m, ts(jh, 512)], qps,
                                             1.0 / W_SCALE)

            # ---- prologue (PE order matches DMA arrival order).  The
            # first two score tiles are emitted per j-half so their j0
            # exps can run while x0-j1 is still in flight. ----
            qT0_sb = stream.tile([128, 2, QB], bf16, tag="qt0", bufs=1,
                                 name="qT0")
            emit_qt_j(qT0_sb, x0_sb, 0, 0, 0, pool=psS, tag="S")
            emit_kt_q(0, 0, psS, "S")
            # early half-score tiles in psT/psO (separate tiles per half so
            # the j0 exps carry no false dependency on the j1 writes)
            early_half = {}
            for g0 in (0, 1):
                early_half[(g0, 0)] = psT.tile([128, 512], f32, tag="T",
                                               name=f"se{g0}a")
                nc.tensor.matmul(
                    early_half[(g0, 0)],
                    kT_sb[0:D, 0, ts(g0, 128)],
                    qT0_sb[0:D, 0, 0:512],
                    start=True, stop=True,
                )
            emit_v_chunk(0)
            emit_qt_j(qT0_sb, x0_sb, 0, 0, 1)
            for g0 in (0, 1):
                early_half[(g0, 1)] = psO.tile([128, 512], f32, tag="O",
                                               name=f"se{g0}b")
                nc.tensor.matmul(
                    early_half[(g0, 1)],
                    kT_sb[0:D, 0, ts(g0, 128)],
                    qT0_sb[0:D, 0, 512:1024],
                    start=True, stop=True,
                )
            emit_v_chunk(1)
            emit_v_chunk(2)
            emit_v_chunk(3)

            qT1_sb = stream.tile([128, 2, QB], bf16, tag="qt1", bufs=1,
                                 name="qT1")
            qT_tiles = [qT0_sb, qT1_sb]

            # ---- filler slot table: fill[(qb, h, kc)] -> list of fns ----
            fill = {}

            def add_fill(qb, h, kc, fn):
                fill.setdefault((qb, h, kc), []).append(fn)

            # Fillers are atomic (open + matmuls + evict) and spaced >=3
            # steps apart so the 2-deep score ring / Act exp queue absorbs
            # each clump before the next.  v chunks in h0 are split halves
            # (psT has no other user there).
            add_fill(0, 0, 0, lambda: emit_kt_q(0, 1, psO, "O"))
            add_fill(0, 0, 3, lambda: emit_kt_q(0, 2, psO, "O"))
            add_fill(0, 0, 6, lambda: emit_kt_q(0, 3, psO, "O"))
            for kc0, slot in ((4, 1), (5, 2), (6, 3), (7, 4), (8, 5), (9, 6),
                              (10, 7), (11, 8), (12, 9), (13, 10), (14, 11),
                              (15, 13)):
                add_fill(0, 0, slot, lambda kc=kc0: emit_v_chunk(kc, 0))
                add_fill(0, 0, slot + 1, lambda kc=kc0: emit_v_chunk(kc, 1))
            # kT(m1, q0..q3) gate S(h2, *): fill h1 slots (ctx all landed).
            add_fill(0, 1, 1, lambda: emit_kt_q(1, 0, psO, "O"))
            add_fill(0, 1, 3, lambda: emit_kt_q(1, 1, psO, "O"))
            add_fill(0, 1, 5, lambda: emit_kt_q(1, 2, psO, "O"))
            add_fill(0, 1, 7, lambda: emit_kt_q(1, 3, psO, "O"))
            # qT(qb0, m1): gates S(h2) (emitted step 30).
            add_fill(0, 1, 9, lambda: emit_qt_j(qT0_sb, x0_sb, 1, 0, 0))
            add_fill(0, 1, 11, lambda: emit_qt_j(qT0_sb, x0_sb, 1, 0, 1))
            # qT(qb1, m0): gates S(qb1, h0) (emitted step 62); x1 by ~28us.
            add_fill(0, 2, 2, lambda: emit_qt_j(qT1_sb, x1_sb, 0, 1, 0))
            add_fill(0, 3, 1, lambda: emit_qt_j(qT1_sb, x1_sb, 0, 1, 1))
            # qT(qb1, m1): gates S(qb1, h2) (emitted step 94): qb1-h0 slots.
            add_fill(1, 0, 1, lambda: emit_qt_j(qT1_sb, x1_sb, 1, 1, 0))
            add_fill(1, 1, 1, lambda: emit_qt_j(qT1_sb, x1_sb, 1, 1, 1))

            # ---- final projection: one (qm, jh) half ----
            ost_tiles = {}
            opool_toggle = {}

            def emit_final_half(qb, qm, jh, pool=None, tag=None,
                                evict_act=False, evict_split=False):
                pool = pool or psO
                tag = tag or "O"
                ops = pool.tile([128, 512], f32, tag=tag,
                                name=f"ops{qb}{qm}{jh}")
                for t in range(2):
                    nc.tensor.matmul(
                        ops,
                        ot_alls[qb][:, t, ts(qm, 128)],
                        wo_sb[:, t, ts(jh, 512)],
                        start=(t == 0),
                        stop=(t == 1),
                    )
                if (qb, qm) not in ost_tiles:
                    ost_tiles[(qb, qm)] = stream.tile(
                        [128, Q_DIM], bf16, tag="ost", bufs=4,
                        name=f"ost{qb}{qm}"
                    )
                ost = ost_tiles[(qb, qm)]
                if evict_split:
                    nc.vector.tensor_copy(out=ost[:, ds(jh * 512, 256)],
                                          in_=ops[:, 0:256])
                    nc.scalar.copy(out=ost[:, ds(jh * 512 + 256, 256)],
                                   in_=ops[:, 256:512])
                elif evict_act:
                    nc.scalar.copy(out=ost[:, ts(jh, 512)], in_=ops)
                else:
                    nc.vector.tensor_copy(out=ost[:, ts(jh, 512)], in_=ops)
                if jh == 1:
                    nc.sync.dma_start(
                        out=out_d[ds(qb * QB + qm * 128, 128), :], in_=ost
                    )

            # ---- attention state ----
            ot_alls = {}
            pv_banks = {}    # (qb, h) -> [tile_a, tile_b]
            pk_tiles = {}    # (qb, pair) -> packed normalized tile
            pts = {}         # step -> pt tile
            sts = {}         # step -> score tile (act) or sbuf copy (pool)

            steps = [(qb, h, kc)
                     for qb in range(N_QB)
                     for h in range(H_PER_CORE)
                     for kc in range(KC)]
            NSTEP = len(steps)

            def is_pool(qb, h, kc):
                if (qb, h) in NO_POOL_HEADS:
                    return kc in (9, 11)
                return kc in POOL_KCS

            def lead_of(qb, h, kc):
                if is_pool(qb, h, kc):
                    return PV_LEAD_POOL
                # stagger the first PVs of each head so they never wait on
                # the previous head's normalize reads (psPV is single-buffered)
                return {0: 4, 1: 3, 2: 2}.get(kc, PV_LEAD_ACT)

            # per-step PV due lists: pv_due[g] = list of source steps
            pv_due = [[] for _ in range(NSTEP + PV_LEAD_POOL + 1)]
            for g, (qb, h, kc) in enumerate(steps):
                pv_due[g + lead_of(qb, h, kc)].append(g)
            # per-(qb,h) bank emission bookkeeping: which (kc, j) is first/
            # last per bank in emission order.
            bank_seq = {}  # (qb, h, bank) -> [positions...] as (kc, j)
            for g, (qb, h, kc) in enumerate(steps):
                for j in range(8):
                    bank_seq.setdefault((qb, h, j // 4), []).append(
                        (g + lead_of(qb, h, kc), kc, j))
            for key in bank_seq:
                bank_seq[key].sort()
            bank_first = {k: v[0] for k, v in bank_seq.items()}
            bank_last = {k: v[-1] for k, v in bank_seq.items()}

            def emit_S(g):
                if g <= 1:
                    return None  # prologue-emitted half tiles
                qb, h, kc = steps[g]
                t, po = h // 2, (h % 2) * D
                if is_pool(qb, h, kc):
                    # pool-path scores live outside the psS ring: one 512
                    # half in psT, one in psO
                    st = (
                        psT.tile([128, 512], f32, tag="T", name=f"sp{qb}{h}{kc}a"),
                        psO.tile([128, 512], f32, tag="O", name=f"sp{qb}{h}{kc}b"),
                    )
                    for j in range(2):
                        nc.tensor.matmul(
                            st[j],
                            kT_sb[po : po + D, t, ts(kc, 128)],
                            qT_tiles[qb][po : po + D, t, ts(j, 512)],
                            start=True,
                            stop=True,
                        )
                    return st
                st = psS.tile([128, QB], f32, tag="S", name=f"st{qb}{h}{kc}")
                for j in range(2):
                    nc.tensor.matmul(
                        st[:, ts(j, 512)],
                        kT_sb[po : po + D, t, ts(kc, 128)],
                        qT_tiles[qb][po : po + D, t, ts(j, 512)],
                        start=True,
                        stop=True,
                    )
                return st

            dbg_pt_tile = {}

            def emit_exp(g):
                if g <= 1:
                    return  # emitted before the loop
                qb, h, kc = steps[g]
                pt = stream.tile([128, QB], bf16, tag="pt", bufs=10,
                                 name=f"pt{qb}{h}{kc}")
                if _DEBUG and g == 0:
                    dbg_pt_tile["t"] = stream.tile([128, QB], bf16, tag="dbgpt",
                                                   bufs=1, name="dbgpt")
                st = sts.pop(g)
                if g <= 1:
                    return  # handled by the early half-exp block
                if is_pool(qb, h, kc):
                    st_sb = stream.tile([128, QB], f32, tag="stsb", bufs=5,
                                        name=f"stsb{qb}{h}{kc}")
                    nc.vector.tensor_copy(out=st_sb[:, 0:512], in_=st[0])
                    nc.vector.tensor_copy(out=st_sb[:, 512:1024], in_=st[1])
                    nc.gpsimd.tensor_tensor(out=pt, in0=base, in1=st_sb,
                                            op=Pow)
                else:
                    nc.scalar.activation(out=pt, in_=st, func=Exp,
                                         scale=SCALE)
                if _DEBUG and g == 0:
                    nc.vector.tensor_copy(out=dbg_pt_tile["t"], in_=pt)
                pts[g] = pt

            def emit_PV(src_g):
                qb, h, kc = steps[src_g]
                if (qb, h) not in pv_banks:
                    pv_banks[(qb, h)] = [
                        psPV.tile([128, 4, D + 1], f32, tag=f"pv{bk}",
                                  name=f"pv{qb}{h}{bk}")
                        for bk in range(2)
                    ]
                pt = pts.pop(src_g)
                banks = pv_banks[(qb, h)]
                for j in range(8):
                    bk = j // 4
                    nc.tensor.matmul(
                        banks[bk][:, j % 4, :],
                        pt[:, ts(j, 128)],
                        v_sb[:, kc, h, :],
                        start=((kc, j) == bank_first[(qb, h, bk)][1:]),
                        stop=((kc, j) == bank_last[(qb, h, bk)][1:]),
                    )

            Copy = mybir.ActivationFunctionType.Copy

            _norm_state = {}

            def emit_norm(qb, h, burst=None):
                """recip + normalize-evict into the packed pair tile, split
                into two 4-mult bursts on consecutive steps so the DVE never
                sees one long burst.  In the tail Act helps (it is idle)."""
                tail = (qb, h) == (N_QB - 1, H_PER_CORE - 1)
                pair, slot = h // 2, h % 2
                if (qb, pair) not in pk_tiles:
                    pk_tiles[(qb, pair)] = stream.tile(
                        [128, 8, 2, D], bf16, tag="pk", bufs=3,
                        name=f"pk{qb}{pair}"
                    )
                pk = pk_tiles[(qb, pair)]
                if burst in (None, 0):
                    banks = pv_banks.pop((qb, h))
                    recip = stream.tile([128, 8], f32, tag="recip", bufs=3,
                                        name=f"rc{qb}{h}")
                    for bk in range(2):
                        nc.vector.reciprocal(out=recip[:, ds(bk * 4, 4)],
                                             in_=banks[bk][:, :, D])
                    _norm_state[(qb, h)] = (banks, recip)
                banks, recip = _norm_state[(qb, h)]
                jlo, jhi = {None: (0, 8), 0: (0, 4), 1: (4, 8)}[burst]
                for j in range(jlo, jhi):
                    if tail and j % 2 == 1:
                        nc.scalar.activation(
                            out=pk[:, j, slot, :],
                            in_=banks[j // 4][:, j % 4, 0:D],
                            func=Copy, scale=recip[:, ds(j, 1)],
                        )
                    else:
                        nc.vector.tensor_scalar_mul(
                            pk[:, j, slot, :],
                            banks[j // 4][:, j % 4, 0:D],
                            recip[:, ds(j, 1)],
                        )
                if burst == 1 or burst is None:
                    _norm_state.pop((qb, h))

            def emit_transpose(qb, pair):
                pk = pk_tiles.pop((qb, pair))
                trp = psT.tile([128, 8, 128], bf16, tag="T",
                               name=f"trp{qb}{pair}")
                for qc in range(8):
                    nc.tensor.matmul(
                        trp[:, qc, :],
                        pk[:, qc, :, :],
                        ident_sb,
                        start=True,
                        stop=True,
                        is_transpose=True,
                    )
                if qb not in ot_alls:
                    ot_alls[qb] = stream.tile([128, 2, QB], bf16,
                                              tag="otall", bufs=2,
                                              name=f"otall{qb}")
                if (qb, pair) == (N_QB - 1, 1):
                    # tail: split across DVE and the now-idle Act so the
                    # first output-projection groups start sooner
                    flat = trp.rearrange("p a b -> p (a b)")
                    nc.vector.tensor_copy(
                        out=ot_alls[qb][:, pair, 0:512], in_=flat[:, 0:512])
                    nc.scalar.copy(
                        out=ot_alls[qb][:, pair, 512:1024],
                        in_=flat[:, 512:1024])
                else:
                    nc.vector.tensor_copy(
                        out=ot_alls[qb][:, pair, :],
                        in_=trp.rearrange("p a b -> p (a b)"),
                    )

            # outproj(qb0) fillers bind to qb1's pool steps: emitted two
            # steps before pool-kc p, the psO tag chain delays their matmuls
            # until poolS(p) is copied, so the Act eviction lands exactly in
            # pool step p's Act idle.
            for qm in range(8):
                for jh in range(2):
                    slot = qm * 2 + jh  # 0..15
                    add_fill(1, slot // 4, POOL_KCS[slot % 4],
                             lambda qm=qm, jh=jh: emit_final_half(
                                 0, qm, jh, evict_act=False))

            pend = {}

            def add_pend(g, fn):
                pend.setdefault(g, []).append(fn)

            # ---- flat software-pipelined stream ----
            # steps 0/1: half-exps in readiness order (j0 halves first)
            for g0 in (0, 1):
                pts[g0] = stream.tile([128, QB], bf16, tag="pt", bufs=10,
                                      name=f"pte{g0}")
            for jh in (0, 1):
                for g0 in (0, 1):
                    nc.scalar.activation(out=pts[g0][:, ts(jh, 512)],
                                         in_=early_half.pop((g0, jh)),
                                         func=Exp, scale=SCALE)
            sts[0] = sts[1] = None
            for g, (qb, h, kc) in enumerate(steps):
                if g + 2 < NSTEP:
                    sts[g + 2] = emit_S(g + 2)
                emit_exp(g)
                for fn in pend.pop(g, ()):
                    fn()
                for src in pv_due[g]:
                    emit_PV(src)
                # head completion: when the last PV emission position for
                # (qb', h') was at this g, normalize (and transpose on pairs)
                for key, (lg, lkc, lj) in list(bank_last.items()):
                    pqb, ph, pbk = key
                    if pbk == 1 and lg == g:
                        if (pqb, ph) == (N_QB - 1, H_PER_CORE - 1):
                            emit_norm(pqb, ph)
                            emit_transpose(pqb, 1)
                        else:
                            emit_norm(pqb, ph, burst=0)
                            add_pend(g + 1, lambda pqb=pqb, ph=ph:
                                     emit_norm(pqb, ph, burst=1))
                            if ph % 2 == 1:
                                # +2 at most: the first outproj filler that
                                # reads the transposed tile sits at the
                                # next head's first pool slot
                                add_pend(g + 3, lambda pqb=pqb, ph=ph:
                                         emit_transpose(pqb, ph // 2))
                for fn in fill.pop((qb, h, kc), ()):
                    fn()

            # ---- drain ----
            for g in range(NSTEP, NSTEP + PV_LEAD_POOL + 1):
                for fn in pend.pop(g, ()):
                    fn()
                for src in pv_due[g]:
                    emit_PV(src)
                for key, (lg, lkc, lj) in list(bank_last.items()):
                    pqb, ph, pbk = key
                    if pbk == 1 and lg == g:
                        emit_norm(pqb, ph)
                        if ph % 2 == 1:
                            emit_transpose(pqb, ph // 2)
            for g in sorted(pend):
                for fn in pend[g]:
                    fn()

            if _DEBUG:
                nc.sync.dma_start(out=dbg_kt[:, :, :], in_=kT_sb)
                nc.sync.dma_start(out=dbg_qt[:, :, :], in_=qT0_sb)
                nc.sync.dma_start(out=dbg_v[:, :, :, :], in_=v_sb)
                nc.sync.dma_start(out=dbg_ot[:, :, :], in_=ot_alls[0])
                nc.sync.dma_start(out=dbg_pt[:, :], in_=dbg_pt_tile["t"])

            # tail: qb1 output projection round-robins over every psum
            # bank that is free after the last normalize, evictions
            # alternating DVE/Act, so the groups and DMAs pipeline ~6 deep.
            tail_pools = [(psS, "S"), (psPV, "pv0"), (psT, "T"),
                          (psS, "S"), (psPV, "pv1"), (psO, "O")]
            for i, (qm, jh) in enumerate(
                    (qm, jh) for qm in range(8) for jh in range(2)):
                pool, tag = tail_pools[i % len(tail_pools)]
                emit_final_half(1, qm, jh, pool=pool, tag=tag,
                                evict_act=(i % 2 == 1))

    nc.finalize()
    return nc


def _get_nc():
    global _CACHED_NC
    if _CACHED_NC is None:
        _CACHED_NC = _build_bass()
    return _CACHED_NC


def _numpy_fallback(x, context, mask, Wq, Wk, Wv, Wout, bout):
    q = (x @ Wq.T).reshape(B, N, H, D)
    k = (context @ Wk.T).reshape(B, M, H, D)
    v = (context @ Wv.T).reshape(B, M, H, D)
    sim = np.einsum("bnhd,bmhd->bhnm", q, k) * SCALE
    sim = np.where(mask[:, None, None, :], sim, -np.finfo(np.float32).max)
    sim -= sim.max(axis=-1, keepdims=True)
    attn = np.exp(sim)
    attn /= attn.sum(axis=-1, keepdims=True)
    out = np.einsum("bhnm,bmhd->bnhd", attn, v).reshape(B, N, INNER)
    return (out @ Wout.T + bout).astype(np.float32)


def kernel(x, context, mask, Wq, Wk, Wv, Wout, bout, _want_results=False):
    import ml_dtypes

    bf = ml_dtypes.bfloat16
    x = np.asarray(x, dtype=np.float32)
    context = np.asarray(context, dtype=np.float32)
    mask = np.asarray(mask)
    Wq = np.asarray(Wq, dtype=np.float32)
    Wk = np.asarray(Wk, dtype=np.float32)
    Wv = np.asarray(Wv, dtype=np.float32)
    Wout = np.asarray(Wout, dtype=np.float32)
    bout = np.asarray(bout, dtype=np.float32)

    if not mask.all():
        return _numpy_fallback(x, context, mask, Wq, Wk, Wv, Wout, bout)

    from concourse.bass_utils import run_bass_kernel_spmd

    f8 = ml_dtypes.float8_e4m3

    def hilo(a, npairs):
        """fp8 hi/lo split in the DoubleRow pair layout [128, npairs, 2, n]."""
        hi = a.astype(f8)
        lo = (a - hi.astype(np.float32)).astype(f8)
        kp = a.shape[0] // 128

        def arr(v):
            return np.ascontiguousarray(
                v.reshape(npairs, 2, 128, a.shape[1]).transpose(2, 0, 1, 3))
        return arr(hi), arr(lo)

    ident = np.eye(128, dtype=np.float32).astype(bf)
    in_maps = []
    for c in range(N_CORES):
        b, hg = c // 2, c % 2
        sl = slice(hg * IN_PER_CORE, (hg + 1) * IN_PER_CORE)
        x8, xr = hilo(np.ascontiguousarray(x[b].T), 4)
        c8, cr = hilo(np.ascontiguousarray(context[b].T), 3)
        wq8, wqr = hilo(np.ascontiguousarray(Wq[sl, :].T) * W_SCALE, 4)
        wk8, wkr = hilo(np.ascontiguousarray(Wk[sl, :].T) * W_SCALE, 3)
        wv8, wvr = hilo(np.ascontiguousarray(Wv[sl, :].T) * W_SCALE, 3)
        in_maps.append(
            {
                "x8": x8, "xr": xr, "c8": c8, "cr": cr,
                "wq8": wq8, "wqr": wqr, "wk8": wk8, "wkr": wkr,
                "wv8": wv8, "wvr": wvr,
                "wo": np.ascontiguousarray(
                    Wout[:, sl].T.reshape(2, 128, 1024).transpose(1, 0, 2)
                ).astype(bf),
                "ident": ident,
            }
        )

    res = run_bass_kernel_spmd(_get_nc(), in_maps, core_ids=list(range(N_CORES)))

    out = np.empty((B, N, Q_DIM), dtype=np.float32)
    for b in range(B):
        out[b] = (res.results[2 * b]["out"].astype(np.float32)
                  + res.results[2 * b + 1]["out"].astype(np.float32) + bout)
    if _want_results:
        return out, res
    return out
